# revision 1
# baseline (speedup 1.0000x reference)
"""Trainium2 Bass kernel for nn_BasicLayer_up (Mamba2D BasicLayer_up block).

Sharding: 8 cores = 4 batches x 2 d_inner-halves. Each core computes all 4
directional Mamba passes for its (batch, d_inner-half); two pairwise
AllReduces per depth stitch the halves (x_proj partials, out_proj partials);
the block tail (norms, bproj, residual) is replicated within each pair.

Device layout: everything is [d-partition, seq-free]; the selective scan runs
as hardware tensor_tensor_scan instructions (one per (state-dim n, d-tile)),
and the y = sum_n C_n * h_n contraction is a strided-write multiply plus an
inner-16 tensor_reduce.
"""

import sys
import numpy as np

sys.path.insert(0, "/opt/trn_rl_repo")

import concourse.bass as bass
import concourse.tile as tile
from concourse import mybir
from concourse.bacc import _bass_rust
from concourse.bass_utils import run_bass_kernel_spmd

F32 = mybir.dt.float32
F16 = mybir.dt.float16
AF = mybir.ActivationFunctionType
OP = mybir.AluOpType

BATCH, HW, DM, DS, DC, DEPTH = 4, 32, 384, 16, 4, 2
DI = 2 * DM          # 768 d_inner
DTR = 24             # dt_rank
L = HW * HW          # 1024
KH = DM // 128       # 3 tiles per d_inner-half / d_model
NC_CORES = 8
EPS = 1e-5
SP = L // 128        # 8 spread columns per stat row

_CACHED = {}


# ---------------------------------------------------------------- perms ----
def _perm_view(ap, dirn):
    """AP view v with v[p, j] = ap[p, P_dirn(j)], shaped [P, HW, HW]."""
    part = ap.ap[0]
    if dirn == 0:
        return bass.AP(tensor=ap.tensor, offset=ap.offset,
                       ap=[part, [HW, HW], [1, HW]])
    if dirn == 1:   # j=(r,c) -> (31-c)*32 + r
        return bass.AP(tensor=ap.tensor, offset=ap.offset + (HW - 1) * HW,
                       ap=[part, [1, HW], [-HW, HW]])
    if dirn == 2:   # reverse
        return bass.AP(tensor=ap.tensor, offset=ap.offset + L - 1,
                       ap=[part, [-HW, HW], [-1, HW]])
    if dirn == 3:   # j=(r,c) -> c*32 + 31 - r
        return bass.AP(tensor=ap.tensor, offset=ap.offset + HW - 1,
                       ap=[part, [-1, HW], [HW, HW]])
    raise ValueError(dirn)


def _r3(ap):
    return ap.rearrange("p (a b) -> p a b", a=HW)


# ------------------------------------------------------------- device ------
def _build_nc():
    nc = bass.Bass()
    dp = nc.declare_dram_parameter

    xT_d = dp("xT", [DM, L], F32, isOutput=False)
    w_inT_d = dp("w_inT", [DEPTH, DM, DI], F32, isOutput=False)
    conv_w_d = dp("conv_w", [DEPTH, DM, DC], F32, isOutput=False)
    conv_b_d = dp("conv_b", [DEPTH, DM, 1], F32, isOutput=False)
    xp_wT_d = dp("xp_wT", [DEPTH, DM, 56], F32, isOutput=False)
    dt_wT_d = dp("dt_wT", [DEPTH, DTR, DM], F32, isOutput=False)
    dt_b_d = dp("dt_b", [DEPTH, DM, 1], F32, isOutput=False)
    A_d = dp("A_half", [DEPTH, DM, DS], F32, isOutput=False)
    D_d = dp("D_half", [DEPTH, DM, 1], F32, isOutput=False)
    mout_wT_d = dp("mout_wT", [DEPTH, DM, DM], F32, isOutput=False)
    bp_wT_d = dp("bp_wT", [DEPTH, DM, DM], F32, isOutput=False)
    mnw_d = dp("mnw", [DEPTH, DM, 1], F32, isOutput=False)
    mnb_d = dp("mnb", [DEPTH, DM, 1], F32, isOutput=False)
    bpb_d = dp("bpb", [DEPTH, DM, 1], F32, isOutput=False)
    lnw_d = dp("lnw", [DEPTH, DM, 1], F32, isOutput=False)
    lnb_d = dp("lnb", [DEPTH, DM, 1], F32, isOutput=False)
    exp_wT_d = dp("exp_wT", [DM, DI], F32, isOutput=False)
    pe_w_d = dp("pe_w", [DI, 1], F32, isOutput=False)
    pe_b_d = dp("pe_b", [DI, 1], F32, isOutput=False)
    membT_d = dp("membT", [2 * KH, 4, 128], F32, isOutput=False)
    ones1_d = dp("ones1", [1, 128], F32, isOutput=False)
    onesK_d = dp("onesK", [128, 1], F32, isOutput=False)
    out_d = dp("out", [DI, L], F32, isOutput=True)

    cc1_in = nc.dram_tensor("cc1_in", [4, 56, L], F32)
    cc1_out = nc.dram_tensor("cc1_out", [4, 56, L], F32)
    cc2_in = nc.dram_tensor("cc2_in", [DM, L], F32)
    cc2_out = nc.dram_tensor("cc2_out", [DM, L], F32)
    srow_d = nc.dram_tensor("srow", [2, L], F32)
    ucst_d = nc.dram_tensor("ucst", [4, DM, L], F32)

    RG = [[0, 1], [2, 3], [4, 5], [6, 7]]

    from contextlib import ExitStack
    with tile.TileContext(nc) as tc, ExitStack() as ctx:
        wpool = ctx.enter_context(tc.tile_pool(name="w", bufs=1))
        big = ctx.enter_context(tc.tile_pool(name="big", bufs=1))
        trans = ctx.enter_context(tc.tile_pool(name="trans", bufs=2))
        bcp = ctx.enter_context(tc.tile_pool(name="bcp", bufs=3))
        hp = ctx.enter_context(tc.tile_pool(name="hp", bufs=1))
        Pp = ctx.enter_context(tc.tile_pool(name="Pp", bufs=1))
        rows = ctx.enter_context(tc.tile_pool(name="rows", bufs=1))
        pmm = ctx.enter_context(tc.tile_pool(name="pmm", bufs=2, space="PSUM"))
        pbc = ctx.enter_context(tc.tile_pool(name="pbc", bufs=1, space="PSUM"))

        def load3(dram, dep, tag, w=None):
            ts = []
            for k in range(KH):
                t = wpool.tile([128, w or dram.shape[2]], F32, tag=f"{tag}{k}",
                               name=f"{tag}{k}")
                nc.sync.dma_start(out=t[:], in_=dram[dep, k * 128:(k + 1) * 128, :])
                ts.append(t)
            return ts

        ones1 = wpool.tile([1, 128], F32)
        nc.sync.dma_start(out=ones1[:], in_=ones1_d[:])
        onesK = wpool.tile([128, 1], F32)
        nc.sync.dma_start(out=onesK[:], in_=onesK_d[:])
        epsb = wpool.tile([128, 1], F32)
        nc.vector.memset(epsb[:], EPS)

        x_sb = [big.tile([128, L], F32, tag=f"x{k}", name=f"x{k}") for k in range(KH)]
        for k in range(KH):
            nc.sync.dma_start(out=x_sb[k][:], in_=xT_d[k * 128:(k + 1) * 128, :])

        def alloc3(tag, dtype=F32):
            return [big.tile([128, L], dtype, tag=f"{tag}{k}", name=f"{tag}{k}")
                    for k in range(KH)]

        def part_ln(src_tiles, nrm_w, nrm_b, dst_tiles):
            """LayerNorm over the partition dim (384 rows over 3 tiles)."""
            s1 = pmm.tile([1, L], F32, tag="ps", name="s1")
            s2 = pmm.tile([1, L], F32, tag="ps", name="s2")
            for k in range(KH):
                sqt = trans.tile([128, L], F32, tag="tmp", name="sqt", bufs=1)
                nc.gpsimd.tensor_tensor(out=sqt[:], in0=src_tiles[k][:],
                                        in1=src_tiles[k][:], op=OP.mult)
                for h in range(2):
                    sl = slice(h * 512, (h + 1) * 512)
                    nc.tensor.matmul(s1[:, sl], onesK[:], src_tiles[k][:, sl],
                                     start=(k == 0), stop=(k == KH - 1))
                    nc.tensor.matmul(s2[:, sl], onesK[:], sqt[:, sl],
                                     start=(k == 0), stop=(k == KH - 1))
            r1 = rows.tile([1, L], F32, tag="r1", name="r1")
            r2 = rows.tile([1, L], F32, tag="r2", name="r2")
            nc.vector.tensor_copy(r1[:], s1[:])
            nc.vector.tensor_copy(r2[:], s2[:])
            nc.sync.dma_start(out=srow_d[0, :], in_=r1[:])
            nc.sync.dma_start(out=srow_d[1, :], in_=r2[:])
            # spread [2, L] dram -> [128, 2, SP]: elem (p, j, i) = srow[j, p*SP+i]
            spr = trans.tile([128, 2 * SP], F32, tag="spr", name="spr")
            nc.sync.dma_start(
                out=spr[:].rearrange("p (a b) -> p a b", a=2),
                in_=bass.AP(tensor=srow_d[:].tensor, offset=0,
                            ap=[[SP, 128], [L, 2], [1, SP]]))
            mu = trans.tile([128, SP], F32, tag="mu", name="mu")
            vv = trans.tile([128, SP], F32, tag="vv", name="vv")
            nc.vector.tensor_scalar_mul(mu[:], spr[:, 0:SP], 1.0 / DM)
            nc.vector.tensor_scalar_mul(vv[:], spr[:, SP:2 * SP], 1.0 / DM)
            mm2 = trans.tile([128, SP], F32, tag="mm2", name="mm2")
            nc.vector.tensor_tensor(out=mm2[:], in0=mu[:], in1=mu[:], op=OP.mult)
            nc.vector.tensor_tensor(out=vv[:], in0=vv[:], in1=mm2[:], op=OP.subtract)
            nc.scalar.activation(vv[:], vv[:], AF.Ln, bias=epsb[:], scale=1.0)
            nc.scalar.activation(vv[:], vv[:], AF.Exp, bias=0.0, scale=-0.5)
            nc.sync.dma_start(out=srow_d[0, :], in_=mu[:])
            nc.sync.dma_start(out=srow_d[1, :], in_=vv[:])
            r3_ = rows.tile([1, L], F32, tag="r1", name="r3_")
            r4_ = rows.tile([1, L], F32, tag="r2", name="r4_")
            nc.sync.dma_start(out=r3_[:], in_=srow_d[0:1, :])
            nc.sync.dma_start(out=r4_[:], in_=srow_d[1:2, :])
            mub = pbc.tile([128, L], F32, tag="mub", name="mub")
            rsb = pbc.tile([128, L], F32, tag="rsb", name="rsb")
            for h in range(2):
                sl = slice(h * 512, (h + 1) * 512)
                nc.tensor.matmul(mub[:, sl], ones1[:], r3_[:, sl], start=True, stop=True)
                nc.tensor.matmul(rsb[:, sl], ones1[:], r4_[:, sl], start=True, stop=True)
            for k in range(KH):
                t1 = trans.tile([128, L], F32, tag="tmp", name="lnt1", bufs=1)
                nc.vector.tensor_tensor(out=t1[:], in0=src_tiles[k][:], in1=mub[:],
                                        op=OP.subtract)
                nc.vector.tensor_tensor(out=t1[:], in0=t1[:], in1=rsb[:], op=OP.mult)
                nc.vector.tensor_scalar(out=dst_tiles[k][:], in0=t1[:],
                                        scalar1=nrm_w[k][:], scalar2=nrm_b[k][:],
                                        op0=OP.mult, op1=OP.add)

        # ================= per-depth =================
        for dep in range(DEPTH):
            w_inT = load3(w_inT_d, dep, "winT")
            conv_w = load3(conv_w_d, dep, "convw")
            conv_b = load3(conv_b_d, dep, "convb")
            xp_wT = load3(xp_wT_d, dep, "xpwT")
            dt_wT = wpool.tile([DTR, DM], F32, tag="dtwT", name="dtwT")
            nc.sync.dma_start(out=dt_wT[:], in_=dt_wT_d[dep])
            dt_b = load3(dt_b_d, dep, "dtb")
            A_sb = load3(A_d, dep, "Ah")
            D_sb = load3(D_d, dep, "Dh")
            mout_wT = load3(mout_wT_d, dep, "moutT")
            bp_wT = load3(bp_wT_d, dep, "bpT")
            mnw = load3(mnw_d, dep, "mnw"); mnb = load3(mnb_d, dep, "mnb")
            bpb = load3(bpb_d, dep, "bpb")
            lnw = load3(lnw_d, dep, "lnw"); lnb = load3(lnb_d, dep, "lnb")

            # ---- in_proj ----
            u_sb = alloc3("u")
            sz_sb = alloc3("sz")
            for e in range(2 * KH):
                pz = pmm.tile([128, L], F32, tag="ps", name="pz")
                for h in range(2):
                    sl = slice(h * 512, (h + 1) * 512)
                    for k in range(KH):
                        nc.tensor.matmul(pz[:, sl], w_inT[k][:, e * 128:(e + 1) * 128],
                                         x_sb[k][:, sl], start=(k == 0), stop=(k == KH - 1))
                if e < KH:
                    nc.vector.tensor_copy(u_sb[e][:], pz[:])
                else:
                    nc.scalar.activation(sz_sb[e - KH][:], pz[:], AF.Silu)

            # ---- phase B: per dir conv + xdb partial ----
            for d in range(4):
                ud = alloc3("ud")
                for k in range(KH):
                    if d == 0:
                        nc.gpsimd.tensor_copy(ud[k][:], u_sb[k][:])
                    else:
                        nc.gpsimd.tensor_copy(_r3(ud[k][:]), _perm_view(u_sb[k][:], d))
                uc = alloc3("uc")
                for k in range(KH):
                    nc.vector.tensor_scalar_mul(uc[k][:], ud[k][:], conv_w[k][:, 0:1])
                    for j in range(1, DC):
                        nc.vector.scalar_tensor_tensor(
                            out=uc[k][:, j:], in0=ud[k][:, :L - j],
                            scalar=conv_w[k][:, j:j + 1], in1=uc[k][:, j:],
                            op0=OP.mult, op1=OP.add)
                    nc.scalar.activation(uc[k][:], uc[k][:], AF.Silu, bias=conv_b[k][:])
                pxdb = pmm.tile([56, L], F32, tag="ps", name="pxdb")
                for h in range(2):
                    sl = slice(h * 512, (h + 1) * 512)
                    for k in range(KH):
                        nc.tensor.matmul(pxdb[:, sl], xp_wT[k][:], uc[k][:, sl],
                                         start=(k == 0), stop=(k == KH - 1))
                xdbp = trans.tile([56, L], F32, tag="xdb", name="xdbp", bufs=1)
                nc.vector.tensor_copy(xdbp[:], pxdb[:])
                nc.sync.dma_start(out=cc1_in[d], in_=xdbp[:])
                for k in range(KH):
                    nc.sync.dma_start(out=ucst_d[d, k * 128:(k + 1) * 128, :], in_=uc[k][:])

            for d in range(4):
                nc.gpsimd.collective_compute("AllReduce", OP.add, replica_groups=RG,
                                             ins=[cc1_in[d]], outs=[cc1_out[d]])

            # ---- phase C: per dir scans ----
            ysum = alloc3("ys")
            for d in range(4):
                uc = alloc3("uc")
                for k in range(KH):
                    nc.sync.dma_start(out=uc[k][:], in_=ucst_d[d, k * 128:(k + 1) * 128, :])
                xdbr = trans.tile([DTR, L], F32, tag="xdb", name="xdbr", bufs=1)
                nc.sync.dma_start(out=xdbr[:], in_=cc1_out[d, 0:DTR, :])
                dt_sb = alloc3("u")       # reuse u slots (dead after phase B)
                dtu = alloc3("ud")        # reuse ud slots
                for k in range(KH):
                    pdt = pmm.tile([128, L], F32, tag="ps", name="pdt")
                    for h in range(2):
                        sl = slice(h * 512, (h + 1) * 512)
                        nc.tensor.matmul(pdt[:, sl], dt_wT[:, k * 128:(k + 1) * 128],
                                         xdbr[:, sl], start=True, stop=True)
                    et = trans.tile([128, L], F32, tag="tmp", name="et", bufs=1)
                    nc.scalar.activation(et[:], pdt[:], AF.Exp, bias=dt_b[k][:])
                    nc.scalar.activation(dt_sb[k][:], et[:], AF.Ln, bias=1.0)
                    nc.gpsimd.tensor_tensor(out=dtu[k][:], in0=dt_sb[k][:],
                                            in1=uc[k][:], op=OP.mult)

                yk3 = [trans.tile([128, L], F32, tag=f"yk{k}", name=f"yk{k}", bufs=1)
                       for k in range(KH)]
                carr3 = [trans.tile([128, DS], F32, tag=f"carr{k}", name=f"carr{k}",
                                    bufs=1) for k in range(KH)]
                for h in range(2):
                    tsl = slice(h * 512, (h + 1) * 512)
                    Pt3 = [Pp.tile([128, 512 * DS], F16, tag=f"P{k}", name=f"P{k}")
                           for k in range(KH)]
                    for n in range(DS):
                        bbc = bcp.tile([128, 512], F32, tag="bc", name="bbc")
                        nc.sync.dma_start(out=bbc[:], in_=bass.AP(
                            tensor=cc1_out[:].tensor,
                            offset=(d * 56 + DTR + n) * L + h * 512,
                            ap=[[0, 128], [1, 512]]))
                        cbc = bcp.tile([128, 512], F32, tag="bc", name="cbc")
                        nc.sync.dma_start(out=cbc[:], in_=bass.AP(
                            tensor=cc1_out[:].tensor,
                            offset=(d * 56 + DTR + DS + n) * L + h * 512,
                            ap=[[0, 128], [1, 512]]))
                        hts = []
                        for k in range(KH):
                            at = trans.tile([128, 512], F32, tag="at", name="at", bufs=2)
                            nc.scalar.activation(at[:], dt_sb[k][:, tsl], AF.Exp,
                                                 scale=A_sb[k][:, n:n + 1])
                            bt = trans.tile([128, 512], F32, tag="bt", name="bt", bufs=2)
                            nc.gpsimd.tensor_tensor(out=bt[:], in0=dtu[k][:, tsl],
                                                    in1=bbc[:], op=OP.mult)
                            ht = hp.tile([128, 512], F32, tag=f"ht{k}", name=f"ht{k}",
                                         bufs=1)
                            init = 0.0 if h == 0 else carr3[k][:, n:n + 1]
                            nc.vector.tensor_tensor_scan(ht[:], at[:], bt[:], init,
                                                         op0=OP.mult, op1=OP.add)
                            if h == 0:
                                nc.scalar.copy(carr3[k][:, n:n + 1], ht[:, 511:512])
                            hts.append(ht)
                        for k in range(KH):
                            nc.vector.tensor_tensor(
                                out=Pt3[k][:, n * 512:(n + 1) * 512], in0=hts[k][:],
                                in1=cbc[:], op=OP.mult)
                    for k in range(KH):
                        rh = trans.tile([128, 512], F32, tag="rh", name="rh", bufs=1)
                        nc.vector.tensor_reduce(
                            rh[:],
                            Pt3[k][:, :512 * (DS // 2)].rearrange(
                                "p (n t) -> p t n", n=DS // 2),
                            axis=mybir.AxisListType.X, op=OP.add)
                        nc.vector.tensor_reduce(
                            yk3[k][:, tsl],
                            Pt3[k][:, 512 * (DS // 2):].rearrange(
                                "p (n t) -> p t n", n=DS // 2),
                            axis=mybir.AxisListType.X, op=OP.add)
                        nc.vector.tensor_tensor(out=yk3[k][:, tsl], in0=yk3[k][:, tsl],
                                                in1=rh[:], op=OP.add)
                for k in range(KH):
                    yk = yk3[k]
                    nc.vector.scalar_tensor_tensor(out=yk[:], in0=uc[k][:],
                                                   scalar=D_sb[k][:, 0:1], in1=yk[:],
                                                   op0=OP.mult, op1=OP.add)
                    if d == 0:
                        nc.gpsimd.tensor_tensor(out=ysum[k][:], in0=yk[:],
                                                in1=sz_sb[k][:], op=OP.mult)
                    else:
                        nc.gpsimd.tensor_tensor(out=_r3(yk[:]), in0=_r3(yk[:]),
                                                in1=_perm_view(sz_sb[k][:], d), op=OP.mult)
                        pv = _perm_view(ysum[k][:], d)
                        nc.gpsimd.tensor_tensor(out=pv, in0=pv, in1=_r3(yk[:]), op=OP.add)

            # ---- out_proj partial + collective 2 ----
            for m in range(KH):
                po = pmm.tile([128, L], F32, tag="ps", name="po")
                for h in range(2):
                    sl = slice(h * 512, (h + 1) * 512)
                    for k in range(KH):
                        nc.tensor.matmul(po[:, sl], mout_wT[k][:, m * 128:(m + 1) * 128],
                                         ysum[k][:, sl], start=(k == 0), stop=(k == KH - 1))
                pm_sb = trans.tile([128, L], F32, tag="yk0", name="pm_sb", bufs=1)
                nc.vector.tensor_copy(pm_sb[:], po[:])
                nc.sync.dma_start(out=cc2_in[m * 128:(m + 1) * 128, :], in_=pm_sb[:])
            nc.gpsimd.collective_compute("AllReduce", OP.add, replica_groups=RG,
                                         ins=[cc2_in[:]], outs=[cc2_out[:]])
            ym = alloc3("ud")   # reuse (dtu dead)
            for k in range(KH):
                nc.sync.dma_start(out=ym[k][:], in_=cc2_out[k * 128:(k + 1) * 128, :])

            # ---- tail ----
            xn = [trans.tile([128, L], F32, tag=f"xn{k}", name=f"xn{k}", bufs=1)
                  for k in range(KH)]
            part_ln(ym, mnw, mnb, xn)
            xb = alloc3("u")   # reuse
            for m in range(KH):
                pb = pmm.tile([128, L], F32, tag="ps", name="pb")
                for h in range(2):
                    sl = slice(h * 512, (h + 1) * 512)
                    for k in range(KH):
                        nc.tensor.matmul(pb[:, sl], bp_wT[k][:, m * 128:(m + 1) * 128],
                                         xn[k][:, sl], start=(k == 0), stop=(k == KH - 1))
                t1 = trans.tile([128, L], F32, tag="tmp", name="resid", bufs=1)
                nc.vector.tensor_scalar(out=t1[:], in0=pb[:], scalar1=bpb[m][:, 0:1],
                                        scalar2=None, op0=OP.add, op1=OP.bypass)
                nc.vector.tensor_tensor(out=xb[m][:], in0=t1[:], in1=x_sb[m][:], op=OP.add)
            part_ln(xb, lnw, lnb, x_sb)

        # ================= PatchExpand =================
        exp_wT = []
        for k in range(KH):
            t = wpool.tile([128, DI], F32, tag=f"winT{k}", name=f"expw{k}")
            nc.sync.dma_start(out=t[:], in_=exp_wT_d[k * 128:(k + 1) * 128, :])
            exp_wT.append(t)
        membT = []
        memb = []
        for e in range(2 * KH):
            t = wpool.tile([4, 128], F32, tag="membT", name=f"membT{e}", bufs=6)
            nc.sync.dma_start(out=t[:], in_=membT_d[e])
            membT.append(t)
            t2 = wpool.tile([128, 4], F32, tag="memb", name=f"memb{e}", bufs=6)
            nc.sync.dma_start(out=t2[:], in_=bass.AP(
                tensor=membT_d[:].tensor, offset=e * 4 * 128,
                ap=[[1, 128], [128, 4]]))
            memb.append(t2)
        pe_w = []
        pe_b = []
        for e in range(2 * KH):
            tw_ = wpool.tile([128, 1], F32, tag="pew", name=f"pew{e}", bufs=6)
            nc.sync.dma_start(out=tw_[:], in_=pe_w_d[e * 128:(e + 1) * 128, :])
            pe_w.append(tw_)
            tb_ = wpool.tile([128, 1], F32, tag="peb", name=f"peb{e}", bufs=6)
            nc.sync.dma_start(out=tb_[:], in_=pe_b_d[e * 128:(e + 1) * 128, :])
            pe_b.append(tb_)

        xe = []
        xe_tags = ["sz0", "sz1", "sz2", "uc0", "uc1", "uc2"]
        for e in range(2 * KH):
            xet = big.tile([128, L], F32, tag=xe_tags[e], name=f"xe{e}")
            pz = pmm.tile([128, L], F32, tag="ps", name="pz2")
            for h in range(2):
                sl = slice(h * 512, (h + 1) * 512)
                for k in range(KH):
                    nc.tensor.matmul(pz[:, sl], exp_wT[k][:, e * 128:(e + 1) * 128],
                                     x_sb[k][:, sl], start=(k == 0), stop=(k == KH - 1))
            nc.vector.tensor_copy(xet[:], pz[:])
            xe.append(xet)

        CQ = DI // 4  # 192
        s1 = pmm.tile([4, L], F32, tag="ps", name="gs1")
        s2 = pmm.tile([4, L], F32, tag="ps", name="gs2")
        for e in range(2 * KH):
            sq = trans.tile([128, L], F32, tag="tmp", name="gsq", bufs=1)
            nc.gpsimd.tensor_tensor(out=sq[:], in0=xe[e][:], in1=xe[e][:], op=OP.mult)
            for h in range(2):
                sl = slice(h * 512, (h + 1) * 512)
                nc.tensor.matmul(s1[:, sl], memb[e][:], xe[e][:, sl],
                                 start=(e == 0), stop=(e == 2 * KH - 1))
                nc.tensor.matmul(s2[:, sl], memb[e][:], sq[:, sl],
                                 start=(e == 0), stop=(e == 2 * KH - 1))
        r1 = rows.tile([4, L], F32, tag="r1", name="gr1")
        r2 = rows.tile([4, L], F32, tag="r2", name="gr2")
        nc.vector.tensor_scalar_mul(r1[:], s1[:], 1.0 / CQ)
        nc.vector.tensor_scalar_mul(r2[:], s2[:], 1.0 / CQ)
        mm2 = trans.tile([4, L], F32, tag="tmp", name="gmm", bufs=1)
        nc.vector.tensor_tensor(out=mm2[:], in0=r1[:], in1=r1[:], op=OP.mult)
        nc.vector.tensor_tensor(out=r2[:], in0=r2[:], in1=mm2[:], op=OP.subtract)
        nc.scalar.activation(r2[:], r2[:], AF.Ln, bias=epsb[0:4, :], scale=1.0)
        nc.scalar.activation(r2[:], r2[:], AF.Exp, bias=0.0, scale=-0.5)
        for e in range(2 * KH):
            mub = pbc.tile([128, L], F32, tag="mub", name="gmub")
            rsb = pbc.tile([128, L], F32, tag="rsb", name="grsb")
            for h in range(2):
                sl = slice(h * 512, (h + 1) * 512)
                nc.tensor.matmul(mub[:, sl], membT[e][:], r1[:, sl], start=True, stop=True)
                nc.tensor.matmul(rsb[:, sl], membT[e][:], r2[:, sl], start=True, stop=True)
            t1 = trans.tile([128, L], F32, tag="tmp", name="gt1", bufs=1)
            nc.vector.tensor_tensor(out=t1[:], in0=xe[e][:], in1=mub[:], op=OP.subtract)
            nc.vector.tensor_tensor(out=t1[:], in0=t1[:], in1=rsb[:], op=OP.mult)
            to = trans.tile([128, L], F32, tag="yk0", name="gto", bufs=1)
            nc.vector.tensor_scalar(out=to[:], in0=t1[:], scalar1=pe_w[e][:, 0:1],
                                    scalar2=pe_b[e][:, 0:1], op0=OP.mult, op1=OP.add)
            nc.sync.dma_start(out=out_d[e * 128:(e + 1) * 128, :], in_=to[:])

    _bass_rust.generate_event_semaphores(nc)
    return nc


# -------------------------------------------------------------- host -------
def _prep_maps(inputs):
    x = np.ascontiguousarray(np.asarray(inputs["x"], dtype=np.float32))
    in_w = np.asarray(inputs["in_proj_w"], dtype=np.float32)
    cw = np.asarray(inputs["conv_w"], dtype=np.float32)
    cb = np.asarray(inputs["conv_b"], dtype=np.float32)
    xp = np.asarray(inputs["x_proj_w"], dtype=np.float32)
    dtw = np.asarray(inputs["dt_w"], dtype=np.float32)
    dtb = np.asarray(inputs["dt_b"], dtype=np.float32)
    A = -np.exp(np.asarray(inputs["A_log"], dtype=np.float32))
    Dp = np.asarray(inputs["D_param"], dtype=np.float32)
    mout = np.asarray(inputs["mout_w"], dtype=np.float32)
    mnw = np.asarray(inputs["mnorm_w"], dtype=np.float32)
    mnb = np.asarray(inputs["mnorm_b"], dtype=np.float32)
    bpw = np.asarray(inputs["bproj_w"], dtype=np.float32)
    bpb = np.asarray(inputs["bproj_b"], dtype=np.float32)
    lnw = np.asarray(inputs["ln_w"], dtype=np.float32)
    lnb = np.asarray(inputs["ln_b"], dtype=np.float32)
    expw = np.asarray(inputs["exp_w"], dtype=np.float32)
    pw = np.asarray(inputs["pe_norm_w"], dtype=np.float32)
    pb = np.asarray(inputs["pe_norm_b"], dtype=np.float32)

    membT = np.zeros((2 * KH, 4, 128), np.float32)
    for e in range(2 * KH):
        for p in range(128):
            membT[e, (e * 128 + p) // (DI // 4), p] = 1.0

    maps = []
    for c in range(NC_CORES):
        b, half = c // 2, c % 2
        sl = slice(half * DM, half * DM + DM)
        m = {
            "xT": np.ascontiguousarray(x[b].T),
            "w_inT": np.ascontiguousarray(np.concatenate(
                [in_w[:, :DI][:, sl], in_w[:, DI:][:, sl]], axis=1).transpose(0, 2, 1)),
            "conv_w": np.ascontiguousarray(cw[:, sl][:, :, ::-1]),
            "conv_b": np.ascontiguousarray(cb[:, sl])[:, :, None],
            "xp_wT": np.ascontiguousarray(xp[:, :, sl].transpose(0, 2, 1)),
            "dt_wT": np.ascontiguousarray(dtw[:, sl].transpose(0, 2, 1)),
            "dt_b": np.ascontiguousarray(dtb[:, sl])[:, :, None],
            "A_half": np.ascontiguousarray(A[:, sl]),
            "D_half": np.ascontiguousarray(Dp[:, sl])[:, :, None],
            "mout_wT": np.ascontiguousarray(mout[:, :, sl].transpose(0, 2, 1)),
            "bp_wT": np.ascontiguousarray(bpw.transpose(0, 2, 1)),
            "mnw": mnw[:, :, None], "mnb": mnb[:, :, None],
            "bpb": bpb[:, :, None],
            "lnw": lnw[:, :, None], "lnb": lnb[:, :, None],
            "exp_wT": np.ascontiguousarray(expw.T),
            "pe_w": np.ascontiguousarray(np.tile(pw, 4))[:, None],
            "pe_b": np.ascontiguousarray(np.tile(pb, 4))[:, None],
            "membT": membT,
            "ones1": np.ones((1, 128), np.float32),
            "onesK": np.ones((128, 1), np.float32),
        }
        maps.append(m)
    return maps


def kernel(**inputs):
    if "nc" not in _CACHED:
        _CACHED["nc"] = _build_nc()
    nc = _CACHED["nc"]
    maps = _prep_maps(inputs)
    import time
    res = None
    for attempt in range(3):
        try:
            res = run_bass_kernel_spmd(nc, maps, core_ids=list(range(NC_CORES)))
            break
        except Exception:
            if attempt == 2:
                raise
            time.sleep(30.0 * (attempt + 1))
    outs = []
    for b in range(BATCH):
        xen = res.results[2 * b]["out"]          # [768, 1024]
        o = xen.reshape(2, 2, DI // 4, HW, HW).transpose(3, 0, 4, 1, 2)
        outs.append(np.ascontiguousarray(o.reshape(2 * HW, 2 * HW, DI // 4)))
    return np.stack(outs).astype(np.float32)



# revision 6
# speedup vs baseline: 1.3839x; 1.3839x over previous
"""Trainium2 Bass kernel for nn_BasicLayer_up (Mamba2D BasicLayer_up block), v2.

Sharding: 8 cores = 4 batches x 2 d_inner-halves (as v1). Rewritten engine
mapping from microbenchmarks:
  - all matmuls fp16 (PE 1 cyc/row vs 4 for f32)
  - depthwise conv = 4 diag-stationary PE matmuls + Silu on Scalar
  - selective-scan inner loop: at=exp(A*dt) on Scalar, bt/P-mult fp16
    tensor_tensor on DVE (2x mode) with a tunable subset of states on GpSimd,
    scan on DVE (1x, ~2.1ns/elem - hard floor), sum over states n via
    identity-stationary PE matmuls accumulating in PSUM, D-skip folded in as a
    diag-stationary matmul.
  - B/C broadcast tiles via DRAM->SBUF stride-0 partition DMA (no engine time)
  - permutes / gates / ysum accumulation: DVE strided views (faster than pool)
"""

import sys
import numpy as np

sys.path.insert(0, "/opt/trn_rl_repo")

import concourse.bass as bass
import concourse.tile as tile
from concourse import mybir
from concourse.bacc import _bass_rust
from concourse.bass_utils import run_bass_kernel_spmd

F32 = mybir.dt.float32
F16 = mybir.dt.float16
AF = mybir.ActivationFunctionType
OP = mybir.AluOpType

BATCH, HW, DM, DS, DC, DEPTH = 4, 32, 384, 16, 4, 2
DI = 2 * DM          # 768 d_inner
DTR = 24             # dt_rank
L = HW * HW          # 1024
KH = DM // 128       # 3 tiles per d_inner-half / d_model
NC_CORES = 8
EPS = 1e-5
SP = L // 128        # 8 spread columns per stat row
PAD = 4              # leading zero columns in conv input tiles
POOL_COUNT = 6       # states whose bt/P-mult run on GpSimd

_CACHED = {}


# ---------------------------------------------------------------- perms ----
def _perm_view(ap, dirn):
    """AP view v with v[p, j] = ap[p, P_dirn(j)], shaped [P, HW, HW]."""
    part = ap.ap[0]
    if dirn == 0:
        return bass.AP(tensor=ap.tensor, offset=ap.offset,
                       ap=[part, [HW, HW], [1, HW]])
    if dirn == 1:   # j=(r,c) -> (31-c)*32 + r
        return bass.AP(tensor=ap.tensor, offset=ap.offset + (HW - 1) * HW,
                       ap=[part, [1, HW], [-HW, HW]])
    if dirn == 2:   # reverse
        return bass.AP(tensor=ap.tensor, offset=ap.offset + L - 1,
                       ap=[part, [-HW, HW], [-1, HW]])
    if dirn == 3:   # j=(r,c) -> c*32 + 31 - r
        return bass.AP(tensor=ap.tensor, offset=ap.offset + HW - 1,
                       ap=[part, [-1, HW], [HW, HW]])
    raise ValueError(dirn)


def _rev_view(ap):
    return bass.AP(tensor=ap.tensor, offset=ap.offset + L - 1,
                   ap=[ap.ap[0], [-1, L]])


def _r3(ap):
    return ap.rearrange("p (a b) -> p a b", a=HW)


# ------------------------------------------------------------- device ------
def _build_nc():
    nc = bass.Bass()
    dp = nc.declare_dram_parameter

    xT_d = dp("xT", [DM, L], F32, isOutput=False)
    w_in_d = dp("w_in16", [DEPTH, DM, DI], F16, isOutput=False)
    cdiag_d = dp("cdiag16", [DEPTH, DC, KH, 128, 128], F16, isOutput=False)
    conv_b_d = dp("conv_b", [DEPTH, DM, 1], F32, isOutput=False)
    xp_w_d = dp("xp_w16", [DEPTH, DM, 56], F16, isOutput=False)
    dt_w_d = dp("dt_w16", [DEPTH, DTR, DM], F16, isOutput=False)
    dt_b_d = dp("dt_b", [DEPTH, DM, 1], F32, isOutput=False)
    A_d = dp("A_half", [DEPTH, DM, DS], F32, isOutput=False)
    ddiag_d = dp("ddiag16", [DEPTH, KH, 128, 128], F16, isOutput=False)
    mout_d = dp("mout16", [DEPTH, DM, DM], F16, isOutput=False)
    bp_d = dp("bp16", [DEPTH, DM, DM], F16, isOutput=False)
    mnw_d = dp("mnw", [DEPTH, DM, 1], F32, isOutput=False)
    mnb_d = dp("mnb", [DEPTH, DM, 1], F32, isOutput=False)
    bpb_d = dp("bpb", [DEPTH, DM, 1], F32, isOutput=False)
    lnw_d = dp("lnw", [DEPTH, DM, 1], F32, isOutput=False)
    lnb_d = dp("lnb", [DEPTH, DM, 1], F32, isOutput=False)
    exp_w_d = dp("exp_w16", [DM, DI], F16, isOutput=False)
    pe_w_d = dp("pe_w", [DI, 1], F32, isOutput=False)
    pe_b_d = dp("pe_b", [DI, 1], F32, isOutput=False)
    membT_d = dp("membT16", [2 * KH, 4, 128], F16, isOutput=False)
    memb_d = dp("memb16", [2 * KH, 128, 4], F16, isOutput=False)
    id16_d = dp("id16", [128, 128], F16, isOutput=False)
    ones1_d = dp("ones1_16", [1, 128], F16, isOutput=False)
    onesK_d = dp("onesK16", [128, 1], F16, isOutput=False)
    out_d = dp("out", [DI, L], F16, isOutput=True)

    cc1_in = nc.dram_tensor("cc1_in", [4, 56, L], F32)
    cc1_out = nc.dram_tensor("cc1_out", [4, 56, L], F32)
    cc2_in = nc.dram_tensor("cc2_in", [DM, L], F32)
    cc2_out = nc.dram_tensor("cc2_out", [DM, L], F32)
    srow_d = nc.dram_tensor("srow", [2, L], F32)
    xdb16_d = nc.dram_tensor("xdb16", [4, 2 * DS, L], F16)

    RG = [[0, 1], [2, 3], [4, 5], [6, 7]]

    from contextlib import ExitStack
    with tile.TileContext(nc) as tc, ExitStack() as ctx:
        wpool = ctx.enter_context(tc.tile_pool(name="w", bufs=1))
        big = ctx.enter_context(tc.tile_pool(name="big", bufs=1))
        rot = ctx.enter_context(tc.tile_pool(name="rot", bufs=3))
        rot2 = ctx.enter_context(tc.tile_pool(name="rot2", bufs=2))
        rotb = ctx.enter_context(tc.tile_pool(name="rotb", bufs=2))
        bcp = ctx.enter_context(tc.tile_pool(name="bcp", bufs=2))
        rows = ctx.enter_context(tc.tile_pool(name="rows", bufs=1))
        pa = ctx.enter_context(tc.tile_pool(name="pa", bufs=2, space="PSUM"))
        py = ctx.enter_context(tc.tile_pool(name="py", bufs=6, space="PSUM"))

        def load3(dram, dep, tag, w=None, dtype=F16):
            ts = []
            for k in range(KH):
                t = wpool.tile([128, w or dram.shape[2]], dtype, tag=f"{tag}{k}",
                               name=f"{tag}{k}")
                nc.sync.dma_start(out=t[:], in_=dram[dep, k * 128:(k + 1) * 128, :])
                ts.append(t)
            return ts

        id16 = wpool.tile([128, 128], F16, tag="id16")
        nc.sync.dma_start(out=id16[:], in_=id16_d[:])
        ones1 = wpool.tile([1, 128], F16, tag="ones1")
        nc.sync.dma_start(out=ones1[:], in_=ones1_d[:])
        onesK = wpool.tile([128, 1], F16, tag="onesK")
        nc.sync.dma_start(out=onesK[:], in_=onesK_d[:])
        epsb = wpool.tile([128, 1], F32, tag="epsb")
        nc.vector.memset(epsb[:], EPS)

        # x master in f16 (3 tiles)
        x32 = [rot2.tile([128, L], F32, tag="st32", name=f"x32_{k}") for k in range(KH)]
        x_sb = [big.tile([128, L], F16, tag=f"x{k}", name=f"x{k}") for k in range(KH)]
        for k in range(KH):
            nc.sync.dma_start(out=x32[k][:], in_=xT_d[k * 128:(k + 1) * 128, :])
        for k in range(KH):
            nc.scalar.activation(x_sb[k][:], x32[k][:], AF.Copy)

        def alloc3(pool, tag, w=L, dtype=F16):
            return [pool.tile([128, w], dtype, tag=f"{tag}{k}", name=f"{tag}{k}")
                    for k in range(KH)]

        # ---------------- partition-dim LayerNorm (f16 elementwise) --------
        def part_ln(src, nrm_w, nrm_b, dst):
            # stats: column sums of x and x^2 over the 384 partition rows
            r1 = rows.tile([1, L], F32, tag="r1", name="r1")
            r2 = rows.tile([1, L], F32, tag="r2", name="r2")
            for h in range(2):
                sl = slice(h * 512, (h + 1) * 512)
                s1 = pa.tile([1, 512], F32, tag="pa", name="s1")
                for k in range(KH):
                    nc.tensor.matmul(s1[:], onesK[:], src[k][:, sl],
                                     start=(k == 0), stop=(k == KH - 1))
                nc.scalar.activation(r1[:, sl], s1[:], AF.Copy)
            for h in range(2):
                sl = slice(h * 512, (h + 1) * 512)
                s2 = pa.tile([1, 512], F32, tag="pa", name="s2")
                for k in range(KH):
                    sq = rot2.tile([128, 512], F16, tag="sq", name="sq")
                    nc.scalar.activation(sq[:], src[k][:, sl], AF.Square)
                    nc.tensor.matmul(s2[:], onesK[:], sq[:],
                                     start=(k == 0), stop=(k == KH - 1))
                nc.scalar.activation(r2[:, sl], s2[:], AF.Copy)
            nc.sync.dma_start(out=srow_d[0, :], in_=r1[:])
            nc.sync.dma_start(out=srow_d[1, :], in_=r2[:])
            # spread [2, L] dram -> [128, 2, SP]
            spr = rot2.tile([128, 2 * SP], F32, tag="spr", name="spr")
            nc.sync.dma_start(
                out=spr[:].rearrange("p (a b) -> p a b", a=2),
                in_=bass.AP(tensor=srow_d[:].tensor, offset=0,
                            ap=[[SP, 128], [L, 2], [1, SP]]))
            mu = rot2.tile([128, SP], F32, tag="mu", name="mu")
            vv = rot2.tile([128, SP], F32, tag="vv", name="vv")
            nc.vector.tensor_scalar_mul(mu[:], spr[:, 0:SP], 1.0 / DM)
            nc.vector.tensor_scalar_mul(vv[:], spr[:, SP:2 * SP], 1.0 / DM)
            mm2 = rot2.tile([128, SP], F32, tag="mm2", name="mm2")
            nc.vector.tensor_tensor(out=mm2[:], in0=mu[:], in1=mu[:], op=OP.mult)
            nc.vector.tensor_tensor(out=vv[:], in0=vv[:], in1=mm2[:], op=OP.subtract)
            nc.scalar.activation(vv[:], vv[:], AF.Ln, bias=epsb[:], scale=1.0)
            nc.scalar.activation(vv[:], vv[:], AF.Exp, bias=0.0, scale=-0.5)
            nc.sync.dma_start(out=srow_d[0, :], in_=mu[:])
            nc.sync.dma_start(out=srow_d[1, :], in_=vv[:])
            r3_ = rows.tile([1, L], F16, tag="r3", name="r3_")
            r4_ = rows.tile([1, L], F16, tag="r4", name="r4_")
            r3f = rows.tile([1, L], F32, tag="r1", name="r3f")
            r4f = rows.tile([1, L], F32, tag="r2", name="r4f")
            nc.sync.dma_start(out=r3f[:], in_=srow_d[0:1, :])
            nc.sync.dma_start(out=r4f[:], in_=srow_d[1:2, :])
            nc.scalar.activation(r3_[:], r3f[:], AF.Copy)
            nc.scalar.activation(r4_[:], r4f[:], AF.Copy)
            mub = big.tile([128, L], F16, tag="mub", name="mub")
            rsb = big.tile([128, L], F16, tag="rsb", name="rsb")
            for h in range(2):
                sl = slice(h * 512, (h + 1) * 512)
                pm = pa.tile([128, 512], F32, tag="pa", name="pmub")
                nc.tensor.matmul(pm[:], ones1[:], r3_[:, sl], start=True, stop=True)
                nc.scalar.activation(mub[:, sl], pm[:], AF.Copy)
                pr = pa.tile([128, 512], F32, tag="pa", name="prsb")
                nc.tensor.matmul(pr[:], ones1[:], r4_[:, sl], start=True, stop=True)
                nc.scalar.activation(rsb[:, sl], pr[:], AF.Copy)
            for k in range(KH):
                xm = rot2.tile([128, L], F16, tag="yg", name="xm")
                nc.vector.tensor_tensor(out=xm[:], in0=src[k][:], in1=mub[:],
                                        op=OP.subtract)
                nc.vector.tensor_tensor(out=xm[:], in0=xm[:], in1=rsb[:], op=OP.mult)
                nc.vector.tensor_scalar(out=dst[k][:], in0=xm[:],
                                        scalar1=nrm_w[k][:, 0:1],
                                        scalar2=nrm_b[k][:, 0:1],
                                        op0=OP.mult, op1=OP.add)

        # ================= per-depth =================
        for dep in range(DEPTH):
            w_in = load3(w_in_d, dep, "win")
            xp_w = load3(xp_w_d, dep, "xpw")
            dt_w = wpool.tile([DTR, DM], F16, tag="dtw", name="dtw")
            nc.sync.dma_start(out=dt_w[:], in_=dt_w_d[dep])
            conv_b = load3(conv_b_d, dep, "convb", dtype=F32)
            dt_b = load3(dt_b_d, dep, "dtb", dtype=F32)
            A_sb = load3(A_d, dep, "Ah", dtype=F32)
            mout_w = load3(mout_d, dep, "mout")
            bp_w = load3(bp_d, dep, "bpw")
            mnw = load3(mnw_d, dep, "mnw", dtype=F32)
            mnb = load3(mnb_d, dep, "mnb", dtype=F32)
            bpb = load3(bpb_d, dep, "bpb", dtype=F32)
            lnw = load3(lnw_d, dep, "lnw", dtype=F32)
            lnb = load3(lnb_d, dep, "lnb", dtype=F32)
            cdiag = [[wpool.tile([128, 128], F16, tag=f"cd{j}{k}", name=f"cd{j}{k}")
                      for k in range(KH)] for j in range(DC)]
            for j in range(DC):
                for k in range(KH):
                    nc.sync.dma_start(out=cdiag[j][k][:], in_=cdiag_d[dep, j, k])
            ddiag = [wpool.tile([128, 128], F16, tag=f"dd{k}", name=f"dd{k}")
                     for k in range(KH)]
            for k in range(KH):
                nc.sync.dma_start(out=ddiag[k][:], in_=ddiag_d[dep, k])

            # ---- in_proj: u into padded ud0 tiles, z -> silu -> sz ----
            ud0 = alloc3(big, "ud0", w=L + PAD)
            sz = alloc3(big, "sz")
            for k in range(KH):
                nc.vector.memset(ud0[k][:, 0:PAD], 0.0)
            for e in range(2 * KH):
                for h in range(2):
                    sl = slice(h * 512, (h + 1) * 512)
                    pz = pa.tile([128, 512], F32, tag="pa", name="pz")
                    for k in range(KH):
                        nc.tensor.matmul(pz[:], w_in[k][:, e * 128:(e + 1) * 128],
                                         x_sb[k][:, sl], start=(k == 0),
                                         stop=(k == KH - 1))
                    if e < KH:
                        nc.scalar.activation(
                            ud0[e][:, PAD + h * 512:PAD + h * 512 + 512], pz[:],
                            AF.Copy)
                    else:
                        nc.scalar.activation(sz[e - KH][:, sl], pz[:], AF.Silu)

            # ---- per dir: permute u, conv (PE diag taps), x_proj, collective
            uc_all = []
            for d in range(4):
                if d == 0:
                    ud = ud0
                else:
                    ud = alloc3(rot2, "ud", w=L + PAD)
                    for k in range(KH):
                        nc.vector.memset(ud[k][:, 0:PAD], 0.0)
                        src_ap = ud0[k][:, PAD:PAD + L]
                        dst_ap = ud[k][:, PAD:PAD + L]
                        if d == 2:
                            nc.scalar.activation(dst_ap, _rev_view(src_ap), AF.Copy)
                        else:
                            nc.vector.tensor_copy(_r3(dst_ap), _perm_view(src_ap, d))
                uc = alloc3(big, f"uc{d}")
                for k in range(KH):
                    for h in range(2):
                        pc = pa.tile([128, 512], F32, tag="pa", name="pc")
                        for j in range(DC):
                            nc.tensor.matmul(
                                pc[:], cdiag[j][k][:],
                                ud[k][:, 1 + h * 512 + j:1 + h * 512 + j + 512],
                                start=(j == 0), stop=(j == DC - 1))
                        nc.scalar.activation(uc[k][:, h * 512:h * 512 + 512], pc[:],
                                             AF.Silu, bias=conv_b[k][:, 0:1])
                uc_all.append(uc)
                xpart = big.tile([56, L], F32, tag="xpart", name="xpart")
                for h in range(2):
                    sl = slice(h * 512, (h + 1) * 512)
                    px = pa.tile([56, 512], F32, tag="pa", name="px")
                    for k in range(KH):
                        nc.tensor.matmul(px[:], xp_w[k][:], uc[k][:, sl],
                                         start=(k == 0), stop=(k == KH - 1))
                    nc.scalar.activation(xpart[:, sl], px[:], AF.Copy)
                nc.sync.dma_start(out=cc1_in[d], in_=xpart[:])
                nc.gpsimd.collective_compute("AllReduce", OP.add, replica_groups=RG,
                                             ins=[cc1_in[d]], outs=[cc1_out[d]])

            # ---- per dir: dt chain + scan inner loop ----
            ysum = alloc3(big, "ys")
            for d in range(4):
                uc = uc_all[d]
                dtr32 = big.tile([DTR, L], F32, tag="dtr32", name="dtr32")
                nc.sync.dma_start(out=dtr32[:], in_=cc1_out[d, 0:DTR, :])
                bc32 = big.tile([2 * DS, L], F32, tag="bc32", name="bc32")
                nc.sync.dma_start(out=bc32[:], in_=cc1_out[d, DTR:56, :])
                dtr16 = rot2.tile([DTR, L], F16, tag="dtr16", name="dtr16")
                nc.scalar.activation(dtr16[:], dtr32[:], AF.Copy)
                xdb16 = rot2.tile([2 * DS, L], F16, tag="xdbs", name="xdbs")
                nc.scalar.activation(xdb16[:], bc32[:], AF.Copy)
                nc.sync.dma_start(out=xdb16_d[d], in_=xdb16[:])

                dt16 = alloc3(big, "dt16")
                for k in range(KH):
                    for h in range(2):
                        sl = slice(h * 512, (h + 1) * 512)
                        pd = pa.tile([128, 512], F32, tag="pa", name="pd")
                        nc.tensor.matmul(pd[:], dt_w[:, k * 128:(k + 1) * 128],
                                         dtr16[:, sl], start=True, stop=True)
                        e16 = rot2.tile([128, 512], F16, tag="e16", name="e16")
                        nc.scalar.activation(e16[:], pd[:], AF.Exp,
                                             bias=dt_b[k][:, 0:1])
                        nc.scalar.activation(dt16[k][:, sl], e16[:], AF.Ln, bias=1.0)
                dtu = alloc3(big, "dtu")
                for k in range(KH):
                    nc.vector.tensor_tensor(out=dtu[k][:], in0=dt16[k][:],
                                            in1=uc[k][:, 0:L], op=OP.mult)

                yp = [[py.tile([128, 512], F32, tag="py", name=f"yp{k}{h}")
                       for h in range(2)] for k in range(KH)]

                def bc_dma(g):
                    t = bcp.tile([128, 2 * L], F16, tag="bc", name=f"bc{g}")
                    nc.sync.dma_start(
                        out=t[:].rearrange("p (a t) -> p a t", a=2),
                        in_=bass.AP(tensor=xdb16_d[:].tensor,
                                    offset=(d * 2 * DS + g) * L,
                                    ap=[[0, 128], [DS * L, 2], [1, L]]))
                    return t

                bct = bc_dma(0)
                for n in range(DS):
                    if n > 0:
                        bct = bc_dma(n)
                    B_view = bct[:, 0:L]
                    C_view = bct[:, L:2 * L]
                    on_pool = n >= DS - POOL_COUNT
                    for k in range(KH):
                        at = rot.tile([128, L], F16, tag="at", name="at")
                        nc.scalar.activation(at[:], dt16[k][:], AF.Exp,
                                             scale=A_sb[k][:, n:n + 1])
                        bt = rot.tile([128, L], F16, tag="bt", name="bt")
                        if on_pool:
                            nc.gpsimd.tensor_tensor(out=bt[:], in0=dtu[k][:],
                                                    in1=B_view, op=OP.mult)
                        else:
                            nc.vector.tensor_tensor(out=bt[:], in0=dtu[k][:],
                                                    in1=B_view, op=OP.mult)
                        ht = rot.tile([128, L], F16, tag="ht", name="ht")
                        nc.vector.tensor_tensor_scan(out=ht[:], data0=at[:],
                                                     data1=bt[:], initial=0.0,
                                                     op0=OP.mult, op1=OP.add)
                        P = rot.tile([128, L], F16, tag="P", name="P")
                        if on_pool:
                            nc.gpsimd.tensor_tensor(out=P[:], in0=ht[:],
                                                    in1=C_view, op=OP.mult)
                        else:
                            nc.vector.tensor_tensor(out=P[:], in0=ht[:],
                                                    in1=C_view, op=OP.mult)
                        for h in range(2):
                            nc.tensor.matmul(yp[k][h][:], id16[:],
                                             P[:, h * 512:h * 512 + 512],
                                             start=(n == 0), stop=False)
                # D-skip fold + gate + accumulate into ysum
                for k in range(KH):
                    y16 = rotb.tile([128, L], F16, tag="y16", name="y16")
                    for h in range(2):
                        nc.tensor.matmul(yp[k][h][:], ddiag[k][:],
                                         uc[k][:, h * 512:h * 512 + 512],
                                         start=False, stop=True)
                        nc.scalar.activation(y16[:, h * 512:h * 512 + 512],
                                             yp[k][h][:], AF.Copy)
                    if d == 0:
                        nc.vector.tensor_tensor(out=ysum[k][:], in0=y16[:],
                                                in1=sz[k][:], op=OP.mult)
                    else:
                        yg = rot2.tile([128, L], F16, tag="yg", name="yg")
                        nc.vector.tensor_tensor(out=_r3(yg[:]), in0=_r3(y16[:]),
                                                in1=_perm_view(sz[k][:], d),
                                                op=OP.mult)
                        pv = _perm_view(ysum[k][:], d)
                        nc.vector.tensor_tensor(out=pv, in0=pv, in1=_r3(yg[:]),
                                                op=OP.add)

            # ---- out_proj partial + collective 2 ----
            for m in range(KH):
                st32 = rot2.tile([128, L], F32, tag="st32", name="st32")
                for h in range(2):
                    sl = slice(h * 512, (h + 1) * 512)
                    po = pa.tile([128, 512], F32, tag="pa", name="po")
                    for k in range(KH):
                        nc.tensor.matmul(po[:], mout_w[k][:, m * 128:(m + 1) * 128],
                                         ysum[k][:, sl], start=(k == 0),
                                         stop=(k == KH - 1))
                    nc.scalar.activation(st32[:, sl], po[:], AF.Copy)
                nc.sync.dma_start(out=cc2_in[m * 128:(m + 1) * 128, :], in_=st32[:])
            nc.gpsimd.collective_compute("AllReduce", OP.add, replica_groups=RG,
                                         ins=[cc2_in[:]], outs=[cc2_out[:]])
            ym16 = alloc3(rotb, "tail16")
            for k in range(KH):
                ym32 = rot2.tile([128, L], F32, tag="st32", name="ym32")
                nc.sync.dma_start(out=ym32[:], in_=cc2_out[k * 128:(k + 1) * 128, :])
                nc.scalar.activation(ym16[k][:], ym32[:], AF.Copy)

            # ---- tail: mnorm -> bproj + residual -> ln ----
            xn = alloc3(rotb, "tail16")
            part_ln(ym16, mnw, mnb, xn)
            xb = alloc3(rotb, "tail16")
            for m in range(KH):
                t16 = rot2.tile([128, L], F16, tag="t16", name="t16")
                for h in range(2):
                    sl = slice(h * 512, (h + 1) * 512)
                    pb = pa.tile([128, 512], F32, tag="pa", name="pb")
                    for k in range(KH):
                        nc.tensor.matmul(pb[:], bp_w[k][:, m * 128:(m + 1) * 128],
                                         xn[k][:, sl], start=(k == 0),
                                         stop=(k == KH - 1))
                    nc.scalar.activation(t16[:, sl], pb[:], AF.Copy)
                t2 = rot2.tile([128, L], F16, tag="t2", name="t2")
                nc.vector.tensor_scalar(out=t2[:], in0=t16[:],
                                        scalar1=bpb[m][:, 0:1], scalar2=None,
                                        op0=OP.add, op1=OP.bypass)
                nc.vector.tensor_tensor(out=xb[m][:], in0=t2[:], in1=x_sb[m][:],
                                        op=OP.add)
            part_ln(xb, lnw, lnb, x_sb)

        # ================= PatchExpand =================
        exp_w = []
        for k in range(KH):
            t = wpool.tile([128, DI], F16, tag=f"win{k}", name=f"expw{k}")
            nc.sync.dma_start(out=t[:], in_=exp_w_d[k * 128:(k + 1) * 128, :])
            exp_w.append(t)
        membT = []
        memb = []
        pe_w = []
        pe_b = []
        for e in range(2 * KH):
            t = wpool.tile([4, 128], F16, tag="membT", name=f"membT{e}", bufs=6)
            nc.sync.dma_start(out=t[:], in_=membT_d[e])
            membT.append(t)
            t2 = wpool.tile([128, 4], F16, tag="memb", name=f"memb{e}", bufs=6)
            nc.sync.dma_start(out=t2[:], in_=memb_d[e])
            memb.append(t2)
            tw_ = wpool.tile([128, 1], F32, tag="pew", name=f"pew{e}", bufs=6)
            nc.sync.dma_start(out=tw_[:], in_=pe_w_d[e * 128:(e + 1) * 128, :])
            pe_w.append(tw_)
            tb_ = wpool.tile([128, 1], F32, tag="peb", name=f"peb{e}", bufs=6)
            nc.sync.dma_start(out=tb_[:], in_=pe_b_d[e * 128:(e + 1) * 128, :])
            pe_b.append(tb_)

        xe = []
        xe_tags = ["sz0", "sz1", "sz2", "ud00", "ud01", "ud02"]
        for e in range(2 * KH):
            xet = big.tile([128, L], F16, tag=xe_tags[e], name=f"xe{e}")
            for h in range(2):
                sl = slice(h * 512, (h + 1) * 512)
                pz = pa.tile([128, 512], F32, tag="pa", name="pz2")
                for k in range(KH):
                    nc.tensor.matmul(pz[:], exp_w[k][:, e * 128:(e + 1) * 128],
                                     x_sb[k][:, sl], start=(k == 0),
                                     stop=(k == KH - 1))
                nc.scalar.activation(xet[:, sl], pz[:], AF.Copy)
            xe.append(xet)

        CQ = DI // 4  # 192
        gr1 = rows.tile([4, L], F32, tag="r1", name="gr1")
        gr2 = rows.tile([4, L], F32, tag="r2", name="gr2")
        for h in range(2):
            sl = slice(h * 512, (h + 1) * 512)
            s1 = pa.tile([4, 512], F32, tag="pa", name="gs1")
            for e in range(2 * KH):
                nc.tensor.matmul(s1[:], memb[e][:], xe[e][:, sl],
                                 start=(e == 0), stop=(e == 2 * KH - 1))
            nc.scalar.activation(gr1[:, sl], s1[:], AF.Copy)
        for h in range(2):
            sl = slice(h * 512, (h + 1) * 512)
            s2 = pa.tile([4, 512], F32, tag="pa", name="gs2")
            for e in range(2 * KH):
                sq = rot2.tile([128, 512], F16, tag="sq", name="gsq")
                nc.scalar.activation(sq[:], xe[e][:, sl], AF.Square)
                nc.tensor.matmul(s2[:], memb[e][:], sq[:],
                                 start=(e == 0), stop=(e == 2 * KH - 1))
            nc.scalar.activation(gr2[:, sl], s2[:], AF.Copy)
        nc.vector.tensor_scalar_mul(gr1[:], gr1[:], 1.0 / CQ)
        nc.vector.tensor_scalar_mul(gr2[:], gr2[:], 1.0 / CQ)
        gmm = rot2.tile([4, L], F32, tag="mm2g", name="gmm")
        nc.vector.tensor_tensor(out=gmm[:], in0=gr1[:], in1=gr1[:], op=OP.mult)
        nc.vector.tensor_tensor(out=gr2[:], in0=gr2[:], in1=gmm[:], op=OP.subtract)
        nc.scalar.activation(gr2[:], gr2[:], AF.Ln, bias=epsb[0:4, :], scale=1.0)
        nc.scalar.activation(gr2[:], gr2[:], AF.Exp, bias=0.0, scale=-0.5)
        gr1_16 = rows.tile([4, L], F16, tag="r3", name="gr1_16")
        gr2_16 = rows.tile([4, L], F16, tag="r4", name="gr2_16")
        nc.scalar.activation(gr1_16[:], gr1[:], AF.Copy)
        nc.scalar.activation(gr2_16[:], gr2[:], AF.Copy)
        for e in range(2 * KH):
            gm = big.tile([128, L], F16, tag="mub", name="gmub")
            gs = big.tile([128, L], F16, tag="rsb", name="grsb")
            for h in range(2):
                sl = slice(h * 512, (h + 1) * 512)
                pm = pa.tile([128, 512], F32, tag="pa", name="gpm")
                nc.tensor.matmul(pm[:], membT[e][:], gr1_16[:, sl], start=True,
                                 stop=True)
                nc.scalar.activation(gm[:, sl], pm[:], AF.Copy)
                pr = pa.tile([128, 512], F32, tag="pa", name="gpr")
                nc.tensor.matmul(pr[:], membT[e][:], gr2_16[:, sl], start=True,
                                 stop=True)
                nc.scalar.activation(gs[:, sl], pr[:], AF.Copy)
            xg = rot2.tile([128, L], F16, tag="yg", name="gxg")
            nc.vector.tensor_tensor(out=xg[:], in0=xe[e][:], in1=gm[:],
                                    op=OP.subtract)
            nc.vector.tensor_tensor(out=xg[:], in0=xg[:], in1=gs[:], op=OP.mult)
            to = rot2.tile([128, L], F16, tag="t16", name="gto")
            nc.vector.tensor_scalar(out=to[:], in0=xg[:], scalar1=pe_w[e][:, 0:1],
                                    scalar2=pe_b[e][:, 0:1], op0=OP.mult, op1=OP.add)
            nc.sync.dma_start(out=out_d[e * 128:(e + 1) * 128, :], in_=to[:])

    _bass_rust.generate_event_semaphores(nc)
    return nc


# -------------------------------------------------------------- host -------
def _prep_maps(inputs):
    x = np.ascontiguousarray(np.asarray(inputs["x"], dtype=np.float32))
    in_w = np.asarray(inputs["in_proj_w"], dtype=np.float32)
    cw = np.asarray(inputs["conv_w"], dtype=np.float32)
    cb = np.asarray(inputs["conv_b"], dtype=np.float32)
    xp = np.asarray(inputs["x_proj_w"], dtype=np.float32)
    dtw = np.asarray(inputs["dt_w"], dtype=np.float32)
    dtb = np.asarray(inputs["dt_b"], dtype=np.float32)
    A = -np.exp(np.asarray(inputs["A_log"], dtype=np.float32))
    Dp = np.asarray(inputs["D_param"], dtype=np.float32)
    mout = np.asarray(inputs["mout_w"], dtype=np.float32)
    mnw = np.asarray(inputs["mnorm_w"], dtype=np.float32)
    mnb = np.asarray(inputs["mnorm_b"], dtype=np.float32)
    bpw = np.asarray(inputs["bproj_w"], dtype=np.float32)
    bpb = np.asarray(inputs["bproj_b"], dtype=np.float32)
    lnw = np.asarray(inputs["ln_w"], dtype=np.float32)
    lnb = np.asarray(inputs["ln_b"], dtype=np.float32)
    expw = np.asarray(inputs["exp_w"], dtype=np.float32)
    pw = np.asarray(inputs["pe_norm_w"], dtype=np.float32)
    pb = np.asarray(inputs["pe_norm_b"], dtype=np.float32)

    membT = np.zeros((2 * KH, 4, 128), np.float16)
    for e in range(2 * KH):
        for p in range(128):
            membT[e, (e * 128 + p) // (DI // 4), p] = 1.0
    memb = np.ascontiguousarray(membT.transpose(0, 2, 1))

    maps = []
    for c in range(NC_CORES):
        b, half = c // 2, c % 2
        sl = slice(half * DM, half * DM + DM)
        cwf = cw[:, sl]  # [DEP, 384, 4] taps, j multiplies u[t+j-3]
        cdiag = np.zeros((DEPTH, DC, KH, 128, 128), np.float16)
        ddiag = np.zeros((DEPTH, KH, 128, 128), np.float16)
        for dep in range(DEPTH):
            for k in range(KH):
                rows_ = slice(k * 128, (k + 1) * 128)
                for j in range(DC):
                    np.fill_diagonal(cdiag[dep, j, k], cwf[dep, rows_, j])
                np.fill_diagonal(ddiag[dep, k], Dp[dep, sl][rows_])
        m = {
            "xT": np.ascontiguousarray(x[b].T),
            "w_in16": np.ascontiguousarray(np.concatenate(
                [in_w[:, :DI][:, sl], in_w[:, DI:][:, sl]],
                axis=1).transpose(0, 2, 1)).astype(np.float16),
            "cdiag16": cdiag,
            "conv_b": np.ascontiguousarray(cb[:, sl])[:, :, None],
            "xp_w16": np.ascontiguousarray(
                xp[:, :, sl].transpose(0, 2, 1)).astype(np.float16),
            "dt_w16": np.ascontiguousarray(
                dtw[:, sl].transpose(0, 2, 1)).astype(np.float16),
            "dt_b": np.ascontiguousarray(dtb[:, sl])[:, :, None],
            "A_half": np.ascontiguousarray(A[:, sl]),
            "ddiag16": ddiag,
            "mout16": np.ascontiguousarray(
                mout[:, :, sl].transpose(0, 2, 1)).astype(np.float16),
            "bp16": np.ascontiguousarray(bpw.transpose(0, 2, 1)).astype(np.float16),
            "mnw": mnw[:, :, None], "mnb": mnb[:, :, None],
            "bpb": bpb[:, :, None],
            "lnw": lnw[:, :, None], "lnb": lnb[:, :, None],
            "exp_w16": np.ascontiguousarray(expw.T).astype(np.float16),
            "pe_w": np.ascontiguousarray(np.tile(pw, 4))[:, None],
            "pe_b": np.ascontiguousarray(np.tile(pb, 4))[:, None],
            "membT16": membT,
            "memb16": memb,
            "id16": np.eye(128, dtype=np.float16),
            "ones1_16": np.ones((1, 128), np.float16),
            "onesK16": np.ones((128, 1), np.float16),
        }
        maps.append(m)
    return maps


def kernel(**inputs):
    if "nc" not in _CACHED:
        _CACHED["nc"] = _build_nc()
    nc = _CACHED["nc"]
    maps = _prep_maps(inputs)
    import time
    res = None
    for attempt in range(3):
        try:
            res = run_bass_kernel_spmd(nc, maps, core_ids=list(range(NC_CORES)))
            break
        except Exception:
            if attempt == 2:
                raise
            time.sleep(30.0 * (attempt + 1))
    outs = []
    for b in range(BATCH):
        xen = res.results[2 * b]["out"].astype(np.float32)   # [768, 1024]
        o = xen.reshape(2, 2, DI // 4, HW, HW).transpose(3, 0, 4, 1, 2)
        outs.append(np.ascontiguousarray(o.reshape(2 * HW, 2 * HW, DI // 4)))
    return np.stack(outs).astype(np.float32)


# revision 8
# speedup vs baseline: 2.0014x; 1.4462x over previous
"""Trainium2 Bass kernel for nn_BasicLayer_up (Mamba2D BasicLayer_up block), v2.

Sharding: 8 cores = 4 batches x 2 d_inner-halves (as v1). Rewritten engine
mapping from microbenchmarks:
  - all matmuls fp16 (PE 1 cyc/row vs 4 for f32)
  - depthwise conv = 4 diag-stationary PE matmuls + Silu on Scalar
  - selective-scan inner loop: at=exp(A*dt) on Scalar, bt/P-mult fp16
    tensor_tensor on DVE (2x mode) with a tunable subset of states on GpSimd,
    scan on DVE (1x, ~2.1ns/elem - hard floor), sum over states n via
    identity-stationary PE matmuls accumulating in PSUM, D-skip folded in as a
    diag-stationary matmul.
  - B/C broadcast tiles via DRAM->SBUF stride-0 partition DMA (no engine time)
  - permutes / gates / ysum accumulation: DVE strided views (faster than pool)
"""

import sys
import numpy as np

sys.path.insert(0, "/opt/trn_rl_repo")

import concourse.bass as bass
import concourse.tile as tile
from concourse import mybir
from concourse.bacc import _bass_rust
from concourse.bass_utils import run_bass_kernel_spmd

F32 = mybir.dt.float32
F16 = mybir.dt.float16
AF = mybir.ActivationFunctionType
OP = mybir.AluOpType

BATCH, HW, DM, DS, DC, DEPTH = 4, 32, 384, 16, 4, 2
DI = 2 * DM          # 768 d_inner
DTR = 24             # dt_rank
L = HW * HW          # 1024
KH = DM // 128       # 3 tiles per d_inner-half / d_model
NC_CORES = 8
EPS = 1e-5
SP = L // 128        # 8 spread columns per stat row
PAD = 4              # leading zero columns in conv input tiles
POOL_COUNT = 6       # states whose bt/P-mult run on GpSimd

_CACHED = {}


# ---------------------------------------------------------------- perms ----
def _perm_view(ap, dirn):
    """AP view v with v[p, j] = ap[p, P_dirn(j)], shaped [P, HW, HW]."""
    part = ap.ap[0]
    if dirn == 0:
        return bass.AP(tensor=ap.tensor, offset=ap.offset,
                       ap=[part, [HW, HW], [1, HW]])
    if dirn == 1:   # j=(r,c) -> (31-c)*32 + r
        return bass.AP(tensor=ap.tensor, offset=ap.offset + (HW - 1) * HW,
                       ap=[part, [1, HW], [-HW, HW]])
    if dirn == 2:   # reverse
        return bass.AP(tensor=ap.tensor, offset=ap.offset + L - 1,
                       ap=[part, [-HW, HW], [-1, HW]])
    if dirn == 3:   # j=(r,c) -> c*32 + 31 - r
        return bass.AP(tensor=ap.tensor, offset=ap.offset + HW - 1,
                       ap=[part, [-1, HW], [HW, HW]])
    raise ValueError(dirn)


def _rev_view(ap):
    return bass.AP(tensor=ap.tensor, offset=ap.offset + L - 1,
                   ap=[ap.ap[0], [-1, L]])


def _r3(ap):
    return ap.rearrange("p (a b) -> p a b", a=HW)


# ------------------------------------------------------------- device ------
def _build_nc():
    nc = bass.Bass()
    dp = nc.declare_dram_parameter

    xT_d = dp("xT", [DM, L], F32, isOutput=False)
    w_in_d = dp("w_in16", [DEPTH, DM, DI], F16, isOutput=False)
    cdiag_d = dp("cdiag16", [DEPTH, DC, KH, 128, 128], F16, isOutput=False)
    conv_b_d = dp("conv_b", [DEPTH, DM, 1], F32, isOutput=False)
    xp_w_d = dp("xp_w16", [DEPTH, DM, 56], F16, isOutput=False)
    dt_w_d = dp("dt_w16", [DEPTH, DTR, DM], F16, isOutput=False)
    dt_b_d = dp("dt_b", [DEPTH, DM, 1], F32, isOutput=False)
    A_d = dp("A_half", [DEPTH, DM, DS], F32, isOutput=False)
    ddiag_d = dp("ddiag16", [DEPTH, KH, 128, 128], F16, isOutput=False)
    mout_d = dp("mout16", [DEPTH, DM, DM], F16, isOutput=False)
    bp_d = dp("bp16", [DEPTH, DM, DM], F16, isOutput=False)
    mnw_d = dp("mnw", [DEPTH, DM, 1], F32, isOutput=False)
    mnb_d = dp("mnb", [DEPTH, DM, 1], F32, isOutput=False)
    bpb_d = dp("bpb", [DEPTH, DM, 1], F32, isOutput=False)
    lnw_d = dp("lnw", [DEPTH, DM, 1], F32, isOutput=False)
    lnb_d = dp("lnb", [DEPTH, DM, 1], F32, isOutput=False)
    exp_w_d = dp("exp_w16", [DM, DI], F16, isOutput=False)
    pe_w_d = dp("pe_w", [DI, 1], F32, isOutput=False)
    pe_b_d = dp("pe_b", [DI, 1], F32, isOutput=False)
    membT_d = dp("membT16", [2 * KH, 4, 128], F16, isOutput=False)
    memb_d = dp("memb16", [2 * KH, 128, 4], F16, isOutput=False)
    id16_d = dp("id16", [128, 128], F16, isOutput=False)
    ones1_d = dp("ones1_16", [1, 128], F16, isOutput=False)
    onesK_d = dp("onesK16", [128, 1], F16, isOutput=False)
    out_d = dp("out", [DI, L], F16, isOutput=True)

    cc1_in = nc.dram_tensor("cc1_in", [4, 56, L], F32)
    cc1_out = nc.dram_tensor("cc1_out", [4, 56, L], F32)
    cc2_in = nc.dram_tensor("cc2_in", [DM, L], F32)
    cc2_out = nc.dram_tensor("cc2_out", [DM, L], F32)
    srow_d = nc.dram_tensor("srow", [2, L], F32)
    xdb16_d = nc.dram_tensor("xdb16", [4, 2 * DS, L], F16)

    RG = [[0, 1], [2, 3], [4, 5], [6, 7]]

    from contextlib import ExitStack
    with tile.TileContext(nc) as tc, ExitStack() as ctx:
        wpool = ctx.enter_context(tc.tile_pool(name="w", bufs=1))
        big = ctx.enter_context(tc.tile_pool(name="big", bufs=1))
        rot = ctx.enter_context(tc.tile_pool(name="rot", bufs=3))
        rot2 = ctx.enter_context(tc.tile_pool(name="rot2", bufs=2))
        rotb = ctx.enter_context(tc.tile_pool(name="rotb", bufs=2))
        bcp = ctx.enter_context(tc.tile_pool(name="bcp", bufs=2))
        rows = ctx.enter_context(tc.tile_pool(name="rows", bufs=1))
        pa = ctx.enter_context(tc.tile_pool(name="pa", bufs=2, space="PSUM"))
        py = ctx.enter_context(tc.tile_pool(name="py", bufs=6, space="PSUM"))

        def load3(dram, dep, tag, w=None, dtype=F16):
            ts = []
            for k in range(KH):
                t = wpool.tile([128, w or dram.shape[2]], dtype, tag=f"{tag}{k}",
                               name=f"{tag}{k}")
                nc.sync.dma_start(out=t[:], in_=dram[dep, k * 128:(k + 1) * 128, :])
                ts.append(t)
            return ts

        id16 = wpool.tile([128, 128], F16, tag="id16")
        nc.sync.dma_start(out=id16[:], in_=id16_d[:])
        ones1 = wpool.tile([1, 128], F16, tag="ones1")
        nc.sync.dma_start(out=ones1[:], in_=ones1_d[:])
        onesK = wpool.tile([128, 1], F16, tag="onesK")
        nc.sync.dma_start(out=onesK[:], in_=onesK_d[:])
        epsb = wpool.tile([128, 1], F32, tag="epsb")
        nc.vector.memset(epsb[:], EPS)

        # x master in f16 (3 tiles)
        x32 = [rot2.tile([128, L], F32, tag="st32", name=f"x32_{k}") for k in range(KH)]
        x_sb = [big.tile([128, L], F16, tag=f"x{k}", name=f"x{k}") for k in range(KH)]
        for k in range(KH):
            nc.sync.dma_start(out=x32[k][:], in_=xT_d[k * 128:(k + 1) * 128, :])
        for k in range(KH):
            nc.scalar.activation(x_sb[k][:], x32[k][:], AF.Copy)

        def alloc3(pool, tag, w=L, dtype=F16):
            return [pool.tile([128, w], dtype, tag=f"{tag}{k}", name=f"{tag}{k}")
                    for k in range(KH)]

        # ---------------- partition-dim LayerNorm (f16 elementwise) --------
        def part_ln(src, nrm_w, nrm_b, dst):
            # stats: column sums of x and x^2 over the 384 partition rows
            r1 = rows.tile([1, L], F32, tag="r1", name="r1")
            r2 = rows.tile([1, L], F32, tag="r2", name="r2")
            for h in range(2):
                sl = slice(h * 512, (h + 1) * 512)
                s1 = pa.tile([1, 512], F32, tag="pa", name="s1")
                for k in range(KH):
                    nc.tensor.matmul(s1[:], onesK[:], src[k][:, sl],
                                     start=(k == 0), stop=(k == KH - 1))
                nc.scalar.activation(r1[:, sl], s1[:], AF.Copy)
            for h in range(2):
                sl = slice(h * 512, (h + 1) * 512)
                s2 = pa.tile([1, 512], F32, tag="pa", name="s2")
                for k in range(KH):
                    sq = rot2.tile([128, 512], F16, tag="e16", name="sq")
                    nc.scalar.activation(sq[:], src[k][:, sl], AF.Square)
                    nc.tensor.matmul(s2[:], onesK[:], sq[:],
                                     start=(k == 0), stop=(k == KH - 1))
                nc.scalar.activation(r2[:, sl], s2[:], AF.Copy)
            # stats -> mean / rstd directly on the [1, L] rows (no DRAM trip)
            r3_ = rows.tile([1, L], F16, tag="r3", name="r3_")
            r4_ = rows.tile([1, L], F16, tag="r4", name="r4_")
            nc.vector.tensor_scalar_mul(r1[:], r1[:], 1.0 / DM)
            nc.vector.tensor_scalar_mul(r2[:], r2[:], 1.0 / DM)
            mm2 = rows.tile([1, L], F32, tag="mm2r", name="mm2r")
            nc.vector.tensor_tensor(out=mm2[:], in0=r1[:], in1=r1[:], op=OP.mult)
            nc.vector.tensor_tensor(out=r2[:], in0=r2[:], in1=mm2[:],
                                    op=OP.subtract)
            nc.scalar.activation(r2[:], r2[:], AF.Ln, bias=epsb[0:1, :], scale=1.0)
            nc.scalar.activation(r2[:], r2[:], AF.Exp, bias=0.0, scale=-0.5)
            nc.scalar.activation(r3_[:], r1[:], AF.Copy)
            nc.scalar.activation(r4_[:], r2[:], AF.Copy)
            mub = big.tile([128, L], F16, tag="mub", name="mub")
            rsb = big.tile([128, L], F16, tag="rsb", name="rsb")
            for h in range(2):
                sl = slice(h * 512, (h + 1) * 512)
                pm = pa.tile([128, 512], F32, tag="pa", name="pmub")
                nc.tensor.matmul(pm[:], ones1[:], r3_[:, sl], start=True, stop=True)
                nc.scalar.activation(mub[:, sl], pm[:], AF.Copy)
                pr = pa.tile([128, 512], F32, tag="pa", name="prsb")
                nc.tensor.matmul(pr[:], ones1[:], r4_[:, sl], start=True, stop=True)
                nc.scalar.activation(rsb[:, sl], pr[:], AF.Copy)
            for k in range(KH):
                xm = rot2.tile([128, L], F16, tag="yg", name="xm")
                nc.vector.tensor_tensor(out=xm[:], in0=src[k][:], in1=mub[:],
                                        op=OP.subtract)
                nc.vector.tensor_tensor(out=xm[:], in0=xm[:], in1=rsb[:], op=OP.mult)
                nc.vector.tensor_scalar(out=dst[k][:], in0=xm[:],
                                        scalar1=nrm_w[k][:, 0:1],
                                        scalar2=nrm_b[k][:, 0:1],
                                        op0=OP.mult, op1=OP.add)

        # ================= per-depth =================
        for dep in range(DEPTH):
            w_in = load3(w_in_d, dep, "win")
            xp_w = load3(xp_w_d, dep, "xpw")
            dt_w = wpool.tile([DTR, DM], F16, tag="dtw", name="dtw")
            nc.sync.dma_start(out=dt_w[:], in_=dt_w_d[dep])
            conv_b = load3(conv_b_d, dep, "convb", dtype=F32)
            dt_b = load3(dt_b_d, dep, "dtb", dtype=F32)
            A_sb = load3(A_d, dep, "Ah", dtype=F32)
            mout_w = load3(mout_d, dep, "mout")
            bp_w = load3(bp_d, dep, "bpw")
            mnw = load3(mnw_d, dep, "mnw", dtype=F32)
            mnb = load3(mnb_d, dep, "mnb", dtype=F32)
            bpb = load3(bpb_d, dep, "bpb", dtype=F32)
            lnw = load3(lnw_d, dep, "lnw", dtype=F32)
            lnb = load3(lnb_d, dep, "lnb", dtype=F32)
            cdiag = [[wpool.tile([128, 128], F16, tag=f"cd{j}{k}", name=f"cd{j}{k}")
                      for k in range(KH)] for j in range(DC)]
            for j in range(DC):
                for k in range(KH):
                    nc.sync.dma_start(out=cdiag[j][k][:], in_=cdiag_d[dep, j, k])
            ddiag = [wpool.tile([128, 128], F16, tag=f"dd{k}", name=f"dd{k}")
                     for k in range(KH)]
            for k in range(KH):
                nc.sync.dma_start(out=ddiag[k][:], in_=ddiag_d[dep, k])

            # ---- in_proj: u into padded ud0 tiles, z -> silu -> sz ----
            ud0 = alloc3(big, "ud0", w=L + PAD)
            sz = alloc3(big, "sz")
            for k in range(KH):
                nc.vector.memset(ud0[k][:, 0:PAD], 0.0)
            for e in range(2 * KH):
                for h in range(2):
                    sl = slice(h * 512, (h + 1) * 512)
                    pz = pa.tile([128, 512], F32, tag="pa", name="pz")
                    for k in range(KH):
                        nc.tensor.matmul(pz[:], w_in[k][:, e * 128:(e + 1) * 128],
                                         x_sb[k][:, sl], start=(k == 0),
                                         stop=(k == KH - 1))
                    if e < KH:
                        nc.scalar.activation(
                            ud0[e][:, PAD + h * 512:PAD + h * 512 + 512], pz[:],
                            AF.Copy)
                    else:
                        nc.scalar.activation(sz[e - KH][:, sl], pz[:], AF.Silu)

            # ---- per dir: permute u, conv (PE diag taps), x_proj, collective
            uc_all = []
            for d in range(4):
                if d == 0:
                    ud = ud0
                else:
                    ud = alloc3(rot2, "ud", w=L + PAD)
                    for k in range(KH):
                        nc.vector.memset(ud[k][:, 0:PAD], 0.0)
                        src_ap = ud0[k][:, PAD:PAD + L]
                        dst_ap = ud[k][:, PAD:PAD + L]
                        if d == 2:
                            nc.scalar.activation(dst_ap, _rev_view(src_ap), AF.Copy)
                        else:
                            nc.vector.tensor_copy(_r3(dst_ap), _perm_view(src_ap, d))
                uc = alloc3(big, f"uc{d}")
                for k in range(KH):
                    for h in range(2):
                        pc = pa.tile([128, 512], F32, tag="pa", name="pc")
                        for j in range(DC):
                            nc.tensor.matmul(
                                pc[:], cdiag[j][k][:],
                                ud[k][:, 1 + h * 512 + j:1 + h * 512 + j + 512],
                                start=(j == 0), stop=(j == DC - 1))
                        nc.scalar.activation(uc[k][:, h * 512:h * 512 + 512], pc[:],
                                             AF.Silu, bias=conv_b[k][:, 0:1])
                uc_all.append(uc)
                xpart = big.tile([56, L], F32, tag="xpart", name="xpart")
                for h in range(2):
                    sl = slice(h * 512, (h + 1) * 512)
                    px = pa.tile([56, 512], F32, tag="pa", name="px")
                    for k in range(KH):
                        nc.tensor.matmul(px[:], xp_w[k][:], uc[k][:, sl],
                                         start=(k == 0), stop=(k == KH - 1))
                    nc.scalar.activation(xpart[:, sl], px[:], AF.Copy)
                nc.sync.dma_start(out=cc1_in[d], in_=xpart[:])
                nc.gpsimd.collective_compute("AllReduce", OP.add, replica_groups=RG,
                                             ins=[cc1_in[d]], outs=[cc1_out[d]])

            # ---- per dir: dt chain + scan inner loop ----
            ysum = alloc3(big, "ys")
            for d in range(4):
                uc = uc_all[d]
                dtr32 = big.tile([DTR, L], F32, tag="dtr32", name="dtr32")
                nc.sync.dma_start(out=dtr32[:], in_=cc1_out[d, 0:DTR, :])
                bc32 = big.tile([2 * DS, L], F32, tag="bc32", name="bc32")
                nc.sync.dma_start(out=bc32[:], in_=cc1_out[d, DTR:56, :])
                dtr16 = rot2.tile([DTR, L], F16, tag="dtr16", name="dtr16")
                nc.scalar.activation(dtr16[:], dtr32[:], AF.Copy)
                xdb16 = big.tile([2 * DS, L], F16, tag="xdbs", name="xdbs")
                nc.scalar.activation(xdb16[:], bc32[:], AF.Copy)
                nc.sync.dma_start(out=xdb16_d[d], in_=xdb16[:])

                dt16 = alloc3(rotb, "dt16")
                for k in range(KH):
                    for h in range(2):
                        sl = slice(h * 512, (h + 1) * 512)
                        pd = pa.tile([128, 512], F32, tag="pa", name="pd")
                        nc.tensor.matmul(pd[:], dt_w[:, k * 128:(k + 1) * 128],
                                         dtr16[:, sl], start=True, stop=True)
                        e16 = rot2.tile([128, 512], F16, tag="e16", name="e16")
                        nc.scalar.activation(e16[:], pd[:], AF.Exp,
                                             bias=dt_b[k][:, 0:1])
                        nc.scalar.activation(dt16[k][:, sl], e16[:], AF.Ln, bias=1.0)
                dtu = alloc3(big, "dtu")
                for k in range(KH):
                    nc.vector.tensor_tensor(out=dtu[k][:], in0=dt16[k][:],
                                            in1=uc[k][:, 0:L], op=OP.mult)

                yp = [[py.tile([128, 512], F32, tag="py", name=f"yp{k}{h}")
                       for h in range(2)] for k in range(KH)]

                def bc_dma(g):
                    t = bcp.tile([128, 2 * L], F16, tag="bc", name=f"bc{g}")
                    nc.sync.dma_start(
                        out=t[:].rearrange("p (a t) -> p a t", a=2),
                        in_=bass.AP(tensor=xdb16_d[:].tensor,
                                    offset=(d * 2 * DS + g) * L,
                                    ap=[[0, 128], [DS * L, 2], [1, L]]))
                    return t

                bct = bc_dma(0)
                for n in range(DS):
                    if n > 0:
                        bct = bc_dma(n)
                    B_view = bct[:, 0:L]
                    C_view = bct[:, L:2 * L]
                    on_pool = n >= DS - POOL_COUNT
                    for k in range(KH):
                        at = rot.tile([128, L], F16, tag="at", name="at")
                        nc.scalar.activation(at[:], dt16[k][:], AF.Exp,
                                             scale=A_sb[k][:, n:n + 1])
                        bt = rot.tile([128, L], F16, tag="bt", name="bt", bufs=2)
                        if on_pool:
                            nc.gpsimd.tensor_tensor(out=bt[:], in0=dtu[k][:],
                                                    in1=B_view, op=OP.mult)
                        else:
                            nc.vector.tensor_tensor(out=bt[:], in0=dtu[k][:],
                                                    in1=B_view, op=OP.mult)
                        ht = rot.tile([128, L], F16, tag="ht", name="ht")
                        nc.vector.tensor_tensor_scan(out=ht[:], data0=at[:],
                                                     data1=bt[:], initial=0.0,
                                                     op0=OP.mult, op1=OP.add)
                        P = rot.tile([128, L], F16, tag="P", name="P", bufs=2)
                        if on_pool:
                            nc.gpsimd.tensor_tensor(out=P[:], in0=ht[:],
                                                    in1=C_view, op=OP.mult)
                        else:
                            nc.vector.tensor_tensor(out=P[:], in0=ht[:],
                                                    in1=C_view, op=OP.mult)
                        for h in range(2):
                            nc.tensor.matmul(yp[k][h][:], id16[:],
                                             P[:, h * 512:h * 512 + 512],
                                             start=(n == 0), stop=False)
                # D-skip fold + gate + accumulate into ysum
                for k in range(KH):
                    y16 = rotb.tile([128, L], F16, tag="y16", name="y16")
                    for h in range(2):
                        nc.tensor.matmul(yp[k][h][:], ddiag[k][:],
                                         uc[k][:, h * 512:h * 512 + 512],
                                         start=False, stop=True)
                        nc.scalar.activation(y16[:, h * 512:h * 512 + 512],
                                             yp[k][h][:], AF.Copy)
                    if d == 0:
                        nc.vector.tensor_tensor(out=ysum[k][:], in0=y16[:],
                                                in1=sz[k][:], op=OP.mult)
                    else:
                        yg = rot2.tile([128, L], F16, tag="yg", name="yg")
                        nc.vector.tensor_tensor(out=_r3(yg[:]), in0=_r3(y16[:]),
                                                in1=_perm_view(sz[k][:], d),
                                                op=OP.mult)
                        pv = _perm_view(ysum[k][:], d)
                        nc.vector.tensor_tensor(out=pv, in0=pv, in1=_r3(yg[:]),
                                                op=OP.add)

            # ---- out_proj partial + collective 2 ----
            for m in range(KH):
                st32 = rot2.tile([128, L], F32, tag="st32", name="st32")
                for h in range(2):
                    sl = slice(h * 512, (h + 1) * 512)
                    po = pa.tile([128, 512], F32, tag="pa", name="po")
                    for k in range(KH):
                        nc.tensor.matmul(po[:], mout_w[k][:, m * 128:(m + 1) * 128],
                                         ysum[k][:, sl], start=(k == 0),
                                         stop=(k == KH - 1))
                    nc.scalar.activation(st32[:, sl], po[:], AF.Copy)
                nc.sync.dma_start(out=cc2_in[m * 128:(m + 1) * 128, :], in_=st32[:])
            nc.gpsimd.collective_compute("AllReduce", OP.add, replica_groups=RG,
                                         ins=[cc2_in[:]], outs=[cc2_out[:]])
            ym16 = alloc3(rotb, "tail16")
            for k in range(KH):
                ym32 = rot2.tile([128, L], F32, tag="st32", name="ym32")
                nc.sync.dma_start(out=ym32[:], in_=cc2_out[k * 128:(k + 1) * 128, :])
                nc.scalar.activation(ym16[k][:], ym32[:], AF.Copy)

            # ---- tail: mnorm -> bproj + residual -> ln ----
            xn = alloc3(rotb, "tail16")
            part_ln(ym16, mnw, mnb, xn)
            xb = alloc3(rotb, "tail16")
            for m in range(KH):
                t16 = rot2.tile([128, L], F16, tag="t16", name="t16")
                for h in range(2):
                    sl = slice(h * 512, (h + 1) * 512)
                    pb = pa.tile([128, 512], F32, tag="pa", name="pb")
                    for k in range(KH):
                        nc.tensor.matmul(pb[:], bp_w[k][:, m * 128:(m + 1) * 128],
                                         xn[k][:, sl], start=(k == 0),
                                         stop=(k == KH - 1))
                    nc.scalar.activation(t16[:, sl], pb[:], AF.Copy)
                t2 = rot2.tile([128, L], F16, tag="yg", name="t2")
                nc.vector.tensor_scalar(out=t2[:], in0=t16[:],
                                        scalar1=bpb[m][:, 0:1], scalar2=None,
                                        op0=OP.add, op1=OP.bypass)
                nc.vector.tensor_tensor(out=xb[m][:], in0=t2[:], in1=x_sb[m][:],
                                        op=OP.add)
            part_ln(xb, lnw, lnb, x_sb)

        # ================= PatchExpand =================
        exp_w = []
        for k in range(KH):
            t = wpool.tile([128, DI], F16, tag=f"win{k}", name=f"expw{k}")
            nc.sync.dma_start(out=t[:], in_=exp_w_d[k * 128:(k + 1) * 128, :])
            exp_w.append(t)
        membT = []
        memb = []
        pe_w = []
        pe_b = []
        for e in range(2 * KH):
            t = wpool.tile([4, 128], F16, tag="membT", name=f"membT{e}", bufs=6)
            nc.sync.dma_start(out=t[:], in_=membT_d[e])
            membT.append(t)
            t2 = wpool.tile([128, 4], F16, tag="memb", name=f"memb{e}", bufs=6)
            nc.sync.dma_start(out=t2[:], in_=memb_d[e])
            memb.append(t2)
            tw_ = wpool.tile([128, 1], F32, tag="pew", name=f"pew{e}", bufs=6)
            nc.sync.dma_start(out=tw_[:], in_=pe_w_d[e * 128:(e + 1) * 128, :])
            pe_w.append(tw_)
            tb_ = wpool.tile([128, 1], F32, tag="peb", name=f"peb{e}", bufs=6)
            nc.sync.dma_start(out=tb_[:], in_=pe_b_d[e * 128:(e + 1) * 128, :])
            pe_b.append(tb_)

        xe = []
        xe_tags = ["sz0", "sz1", "sz2", "ud00", "ud01", "ud02"]
        for e in range(2 * KH):
            xet = big.tile([128, L], F16, tag=xe_tags[e], name=f"xe{e}")
            for h in range(2):
                sl = slice(h * 512, (h + 1) * 512)
                pz = pa.tile([128, 512], F32, tag="pa", name="pz2")
                for k in range(KH):
                    nc.tensor.matmul(pz[:], exp_w[k][:, e * 128:(e + 1) * 128],
                                     x_sb[k][:, sl], start=(k == 0),
                                     stop=(k == KH - 1))
                nc.scalar.activation(xet[:, sl], pz[:], AF.Copy)
            xe.append(xet)

        CQ = DI // 4  # 192
        gr1 = rows.tile([4, L], F32, tag="r1", name="gr1")
        gr2 = rows.tile([4, L], F32, tag="r2", name="gr2")
        for h in range(2):
            sl = slice(h * 512, (h + 1) * 512)
            s1 = pa.tile([4, 512], F32, tag="pa", name="gs1")
            for e in range(2 * KH):
                nc.tensor.matmul(s1[:], memb[e][:], xe[e][:, sl],
                                 start=(e == 0), stop=(e == 2 * KH - 1))
            nc.scalar.activation(gr1[:, sl], s1[:], AF.Copy)
        for h in range(2):
            sl = slice(h * 512, (h + 1) * 512)
            s2 = pa.tile([4, 512], F32, tag="pa", name="gs2")
            for e in range(2 * KH):
                sq = rot2.tile([128, 512], F16, tag="e16", name="gsq")
                nc.scalar.activation(sq[:], xe[e][:, sl], AF.Square)
                nc.tensor.matmul(s2[:], memb[e][:], sq[:],
                                 start=(e == 0), stop=(e == 2 * KH - 1))
            nc.scalar.activation(gr2[:, sl], s2[:], AF.Copy)
        nc.vector.tensor_scalar_mul(gr1[:], gr1[:], 1.0 / CQ)
        nc.vector.tensor_scalar_mul(gr2[:], gr2[:], 1.0 / CQ)
        gmm = rot2.tile([4, L], F32, tag="mm2g", name="gmm")
        nc.vector.tensor_tensor(out=gmm[:], in0=gr1[:], in1=gr1[:], op=OP.mult)
        nc.vector.tensor_tensor(out=gr2[:], in0=gr2[:], in1=gmm[:], op=OP.subtract)
        nc.scalar.activation(gr2[:], gr2[:], AF.Ln, bias=epsb[0:4, :], scale=1.0)
        nc.scalar.activation(gr2[:], gr2[:], AF.Exp, bias=0.0, scale=-0.5)
        gr1_16 = rows.tile([4, L], F16, tag="r3", name="gr1_16")
        gr2_16 = rows.tile([4, L], F16, tag="r4", name="gr2_16")
        nc.scalar.activation(gr1_16[:], gr1[:], AF.Copy)
        nc.scalar.activation(gr2_16[:], gr2[:], AF.Copy)
        for e in range(2 * KH):
            gm = big.tile([128, L], F16, tag="mub", name="gmub")
            gs = big.tile([128, L], F16, tag="rsb", name="grsb")
            for h in range(2):
                sl = slice(h * 512, (h + 1) * 512)
                pm = pa.tile([128, 512], F32, tag="pa", name="gpm")
                nc.tensor.matmul(pm[:], membT[e][:], gr1_16[:, sl], start=True,
                                 stop=True)
                nc.scalar.activation(gm[:, sl], pm[:], AF.Copy)
                pr = pa.tile([128, 512], F32, tag="pa", name="gpr")
                nc.tensor.matmul(pr[:], membT[e][:], gr2_16[:, sl], start=True,
                                 stop=True)
                nc.scalar.activation(gs[:, sl], pr[:], AF.Copy)
            xg = rot2.tile([128, L], F16, tag="yg", name="gxg")
            nc.vector.tensor_tensor(out=xg[:], in0=xe[e][:], in1=gm[:],
                                    op=OP.subtract)
            nc.vector.tensor_tensor(out=xg[:], in0=xg[:], in1=gs[:], op=OP.mult)
            to = rot2.tile([128, L], F16, tag="t16", name="gto")
            nc.vector.tensor_scalar(out=to[:], in0=xg[:], scalar1=pe_w[e][:, 0:1],
                                    scalar2=pe_b[e][:, 0:1], op0=OP.mult, op1=OP.add)
            nc.sync.dma_start(out=out_d[e * 128:(e + 1) * 128, :], in_=to[:])

    _bass_rust.generate_event_semaphores(nc)
    return nc


# -------------------------------------------------------------- host -------
def _prep_maps(inputs):
    x = np.ascontiguousarray(np.asarray(inputs["x"], dtype=np.float32))
    in_w = np.asarray(inputs["in_proj_w"], dtype=np.float32)
    cw = np.asarray(inputs["conv_w"], dtype=np.float32)
    cb = np.asarray(inputs["conv_b"], dtype=np.float32)
    xp = np.asarray(inputs["x_proj_w"], dtype=np.float32)
    dtw = np.asarray(inputs["dt_w"], dtype=np.float32)
    dtb = np.asarray(inputs["dt_b"], dtype=np.float32)
    A = -np.exp(np.asarray(inputs["A_log"], dtype=np.float32))
    Dp = np.asarray(inputs["D_param"], dtype=np.float32)
    mout = np.asarray(inputs["mout_w"], dtype=np.float32)
    mnw = np.asarray(inputs["mnorm_w"], dtype=np.float32)
    mnb = np.asarray(inputs["mnorm_b"], dtype=np.float32)
    bpw = np.asarray(inputs["bproj_w"], dtype=np.float32)
    bpb = np.asarray(inputs["bproj_b"], dtype=np.float32)
    lnw = np.asarray(inputs["ln_w"], dtype=np.float32)
    lnb = np.asarray(inputs["ln_b"], dtype=np.float32)
    expw = np.asarray(inputs["exp_w"], dtype=np.float32)
    pw = np.asarray(inputs["pe_norm_w"], dtype=np.float32)
    pb = np.asarray(inputs["pe_norm_b"], dtype=np.float32)

    membT = np.zeros((2 * KH, 4, 128), np.float16)
    for e in range(2 * KH):
        for p in range(128):
            membT[e, (e * 128 + p) // (DI // 4), p] = 1.0
    memb = np.ascontiguousarray(membT.transpose(0, 2, 1))

    maps = []
    for c in range(NC_CORES):
        b, half = c // 2, c % 2
        sl = slice(half * DM, half * DM + DM)
        cwf = cw[:, sl]  # [DEP, 384, 4] taps, j multiplies u[t+j-3]
        cdiag = np.zeros((DEPTH, DC, KH, 128, 128), np.float16)
        ddiag = np.zeros((DEPTH, KH, 128, 128), np.float16)
        for dep in range(DEPTH):
            for k in range(KH):
                rows_ = slice(k * 128, (k + 1) * 128)
                for j in range(DC):
                    np.fill_diagonal(cdiag[dep, j, k], cwf[dep, rows_, j])
                np.fill_diagonal(ddiag[dep, k], Dp[dep, sl][rows_])
        m = {
            "xT": np.ascontiguousarray(x[b].T),
            "w_in16": np.ascontiguousarray(np.concatenate(
                [in_w[:, :DI][:, sl], in_w[:, DI:][:, sl]],
                axis=1).transpose(0, 2, 1)).astype(np.float16),
            "cdiag16": cdiag,
            "conv_b": np.ascontiguousarray(cb[:, sl])[:, :, None],
            "xp_w16": np.ascontiguousarray(
                xp[:, :, sl].transpose(0, 2, 1)).astype(np.float16),
            "dt_w16": np.ascontiguousarray(
                dtw[:, sl].transpose(0, 2, 1)).astype(np.float16),
            "dt_b": np.ascontiguousarray(dtb[:, sl])[:, :, None],
            "A_half": np.ascontiguousarray(A[:, sl]),
            "ddiag16": ddiag,
            "mout16": np.ascontiguousarray(
                mout[:, :, sl].transpose(0, 2, 1)).astype(np.float16),
            "bp16": np.ascontiguousarray(bpw.transpose(0, 2, 1)).astype(np.float16),
            "mnw": mnw[:, :, None], "mnb": mnb[:, :, None],
            "bpb": bpb[:, :, None],
            "lnw": lnw[:, :, None], "lnb": lnb[:, :, None],
            "exp_w16": np.ascontiguousarray(expw.T).astype(np.float16),
            "pe_w": np.ascontiguousarray(np.tile(pw, 4))[:, None],
            "pe_b": np.ascontiguousarray(np.tile(pb, 4))[:, None],
            "membT16": membT,
            "memb16": memb,
            "id16": np.eye(128, dtype=np.float16),
            "ones1_16": np.ones((1, 128), np.float16),
            "onesK16": np.ones((128, 1), np.float16),
        }
        maps.append(m)
    return maps


def kernel(**inputs):
    if "nc" not in _CACHED:
        _CACHED["nc"] = _build_nc()
    nc = _CACHED["nc"]
    maps = _prep_maps(inputs)
    import time
    res = None
    for attempt in range(3):
        try:
            res = run_bass_kernel_spmd(nc, maps, core_ids=list(range(NC_CORES)))
            break
        except Exception:
            if attempt == 2:
                raise
            time.sleep(30.0 * (attempt + 1))
    outs = []
    for b in range(BATCH):
        xen = res.results[2 * b]["out"].astype(np.float32)   # [768, 1024]
        o = xen.reshape(2, 2, DI // 4, HW, HW).transpose(3, 0, 4, 1, 2)
        outs.append(np.ascontiguousarray(o.reshape(2 * HW, 2 * HW, DI // 4)))
    return np.stack(outs).astype(np.float32)
